# revision 9
# baseline (speedup 1.0000x reference)
"""Single-head self-attention (B=4, S=2048, D=1024) on 8 trn2 NeuronCores.

Sharding: core c -> (batch b = c//2, query half h = c%2). Each core gets a
permuted x^T for its batch (own seq-half first; softmax is invariant to key
permutation), computes Q for its 1024 queries and K/V for all 2048 keys,
then attention. Output rows are the core's own queries in original order, so
the host gather is a pure concatenation.

Device layout (per core):
  xt (input)  : [1024_d, 2048_t] fp32   (x_perm.T, host-prepared)
  Q^T         : [128_dp, 8_dc, 512_s]   per s-block, SBUF
  K^T         : [128_jp, 8_jc, 2048_t]  spilled to HBM scratch, streamed back
  V           : [128_tp, 16_tc, 1024_j] SBUF-resident
  scores^T    : [128_t, 512_s] PSUM -> exp -> SBUF (keys on partitions, so
                attn@V needs no transposes; softmax sum over keys is an
                extra N=1 matmul vs a ones vector sharing the stationary
                operand; max-subtraction skipped: scores ~ N(0, 0.33))
All matmuls fp32r (fp22 mantissa, full PE rate at N>=256, fp32 accumulate).
"""

import os
import sys
import types

import numpy as np

B, S, D = 4, 2048, 1024
HALF = S // 2  # 1024 queries per core
SCALE = 1.0 / 32.0  # 1/sqrt(D)
NC = 8
DC = D // 128  # 8 d-chunks
TT = S // 128  # 16 key tiles
SBLK = 512  # queries per s-block
NSB = HALF // SBLK  # 2 s-blocks

_CACHED_NC = None
LAST_RESULT = None  # BassKernelResults of the most recent run (for test.py)


def _ensure_axon_ntff_hook():
    """bass_utils' trace path needs antenv.axon_hooks; this image's antenv
    lacks it. Install a shim backed by trn_agent_boot's ctypes hook so
    BASS_TRACE=1 profiling works. No-op if already present/unavailable."""
    try:
        import antenv.axon_hooks  # noqa: F401

        return
    except ImportError:
        pass
    try:
        from trn_agent_boot.trn_boot import _ntff_profile_via_ctypes

        hook = _ntff_profile_via_ctypes("/opt/axon/libaxon_pjrt.so")
    except Exception:
        hook = None
    mod = types.ModuleType("antenv.axon_hooks")
    mod.get_axon_ntff_profile_hook = lambda: hook
    mod.set_axon_ntff_profile_hook = lambda h: None
    sys.modules["antenv.axon_hooks"] = mod


def build_kernel(tc, xt, wq, wk, wv, bq, bk, bv, out):
    import concourse.bass as bass
    from concourse import mybir

    nc = tc.nc
    F32 = mybir.dt.float32
    F32R = mybir.dt.float32r
    Identity = mybir.ActivationFunctionType.Identity
    Exp = mybir.ActivationFunctionType.Exp
    PAIRS = [[0, 1], [2, 3], [4, 5], [6, 7]]

    xt_r = xt.rearrange("(c p) t -> p c t", p=128)  # [128, 8, 1024] own half
    out_r = out.rearrange("(su p) j -> su p j", p=128)  # [8, 128, 1024]
    HT = TT // 2  # 8 own key tiles

    with (
        tc.tile_pool(name="persist", bufs=1) as persist,
        tc.tile_pool(name="dram", bufs=1, space="DRAM") as dram,
    ):
        V_sb = persist.tile([128, TT, D], F32R)
        qT = persist.tile([128, DC, HALF], F32R)
        bv_bc = persist.tile([128, D], F32)
        bv_bcast_ap = bass.AP(
            tensor=bv.tensor, offset=bv.offset, ap=[[0, 128]] + list(bv.ap)
        )
        nc.scalar.dma_start(bv_bc, bv_bcast_ap)
        bq_sb = persist.tile([128, DC], F32)
        nc.scalar.dma_start(bq_sb, bq)
        bk_sb = persist.tile([128, DC], F32)
        nc.scalar.dma_start(bk_sb, bk)
        ones_f = persist.tile([128, 2], F32)
        nc.vector.memset(ones_f, 1.0)
        ones_t = persist.tile([128, 2], F32R)
        nc.vector.tensor_copy(ones_t, ones_f)

        kT_own = dram.tile([128, DC, HALF], F32R)
        kT_all = dram.tile([2, 128, DC, HALF], F32R)
        v_own = dram.tile([128, HT, D], F32R)
        v_all = dram.tile([2, 128, HT, D], F32R)

        # ---- Phase A: Q^T, K^T own, V own; pairwise AllGather K^T and V --
        with (
            tc.tile_pool(name="pa", bufs=1) as pa,
            tc.tile_pool(name="pa_w", bufs=3) as paw,
            tc.tile_pool(name="pa_st", bufs=2) as past,
            tc.tile_pool(name="psa", bufs=2, space="PSUM") as psa,
        ):
            # own-half x^T, loaded per-chunk so matmuls start as data lands
            xT = pa.tile([128, DC, HALF], F32R)
            for c in range(DC):
                nc.sync.dma_start(xT[:, c, :], xt_r[:, c, :])

            # Q^T[dq, s] = sum_d Wq[d, dq] xT[d, s]  (+bq fused on copy-out)
            for qc in range(DC):
                wq_t = paw.tile([128, DC, 128], F32R, tag="w_t")
                nc.scalar.dma_start(wq_t, wq[:, :, qc * 128 : (qc + 1) * 128])
                for sblk in range(NSB):
                    qpsum = psa.tile([128, SBLK], F32, tag="qpsum")
                    for c in range(DC):
                        nc.tensor.matmul(
                            qpsum,
                            wq_t[:, c, :],
                            xT[:, c, sblk * SBLK : (sblk + 1) * SBLK],
                            start=(c == 0),
                            stop=(c == DC - 1),
                        )
                    nc.scalar.activation(
                        qT[:, qc, sblk * SBLK : (sblk + 1) * SBLK],
                        qpsum,
                        Identity,
                        bias=bq_sb[:, qc : qc + 1],
                    )

            # K^T own half (+bk) -> DRAM, then pairwise AllGather
            for jt in range(DC):
                wk_t = paw.tile([128, DC, 128], F32R, tag="w_t")
                nc.scalar.dma_start(wk_t, wk[:, :, jt * 128 : (jt + 1) * 128])
                for tb in range(HALF // 512):
                    kpsum = psa.tile([128, 512], F32, tag="kpsum")
                    for c in range(DC):
                        nc.tensor.matmul(
                            kpsum,
                            wk_t[:, c, :],
                            xT[:, c, tb * 512 : (tb + 1) * 512],
                            start=(c == 0),
                            stop=(c == DC - 1),
                        )
                    kstage = past.tile([128, 512], F32R, tag="kstage")
                    nc.scalar.activation(
                        kstage, kpsum, Identity, bias=bk_sb[:, jt : jt + 1]
                    )
                    nc.sync.dma_start(
                        kT_own[:, jt, tb * 512 : (tb + 1) * 512], kstage
                    )
            nc.gpsimd.collective_compute(
                "AllGather",
                mybir.AluOpType.bypass,
                replica_groups=PAIRS,
                ins=[kT_own.opt()],
                outs=[kT_all.opt()],
            )

            # V own half (+bv) -> DRAM, AllGather, land both halves in SBUF
            for jb in range(2):
                wv_h = pa.tile([128, DC, 512], F32R, tag="wv_h", bufs=2)
                nc.scalar.dma_start(wv_h, wv[:, :, jb * 512 : (jb + 1) * 512])
                for tt in range(HT):
                    vpsum = psa.tile([128, 512], F32, tag="vpsum")
                    for c in range(DC):
                        nc.tensor.matmul(
                            vpsum,
                            xT[:, c, tt * 128 : (tt + 1) * 128],
                            wv_h[:, c, :],
                            start=(c == 0),
                            stop=(c == DC - 1),
                        )
                    vstage = past.tile([128, 512], F32R, tag="vstage")
                    nc.vector.tensor_add(
                        vstage, vpsum, bv_bc[:, jb * 512 : (jb + 1) * 512]
                    )
                    nc.sync.dma_start(
                        v_own[:, tt, jb * 512 : (jb + 1) * 512], vstage
                    )
            nc.gpsimd.collective_compute(
                "AllGather",
                mybir.AluOpType.bypass,
                replica_groups=PAIRS,
                ins=[v_own.opt()],
                outs=[v_all.opt()],
            )
            nc.scalar.dma_start(V_sb[:, 0:HT, :], v_all[0])
            nc.scalar.dma_start(V_sb[:, HT:TT, :], v_all[1])

        # ------------- Phase B: scores^T, softmax, out --------------------
        with (
            tc.tile_pool(name="pb_kt", bufs=2) as pbkt,
            tc.tile_pool(name="pb_p", bufs=1) as pbp,
            tc.tile_pool(name="pb_o", bufs=2) as pbo,
            tc.tile_pool(name="pb_m", bufs=2) as pbm,
            tc.tile_pool(name="psb_s", bufs=2, space="PSUM") as psbs,
            tc.tile_pool(name="psb_o", bufs=2, space="PSUM") as psbo,
            tc.tile_pool(name="psb_l", bufs=2, space="PSUM") as psbl,
        ):
            # scores^T + exp for both s-blocks per K^T tile (K^T read once)
            expP = pbp.tile([128, TT, HALF], F32R, tag="expP")
            for tt in range(TT):
                r_, lt = divmod(tt, HT)
                kt_t = pbkt.tile([128, DC, 128], F32R, tag="kt_t")
                nc.sync.dma_start(
                    kt_t, kT_all[r_, :, :, lt * 128 : (lt + 1) * 128]
                )
                for sb in range(NSB):
                    spsum = psbs.tile([128, SBLK], F32, tag="spsum")
                    for jc in range(DC):
                        nc.tensor.matmul(
                            spsum,
                            kt_t[:, jc, :],
                            qT[:, jc, sb * SBLK : (sb + 1) * SBLK],
                            start=(jc == 0),
                            stop=(jc == DC - 1),
                        )
                    nc.scalar.activation(
                        expP[:, tt, sb * SBLK : (sb + 1) * SBLK],
                        spsum,
                        Exp,
                        scale=SCALE,
                    )

            # E[p, s] = sum_tt expP[p, tt, s] via a DVE accumulate chain;
            # the softmax sum l[s] then needs only one matmul per query tile.
            E_t = pbp.tile([128, HALF], F32R, tag="E_t")
            nc.vector.tensor_add(E_t, expP[:, 0, :], expP[:, 1, :])
            for tt in range(2, TT):
                nc.vector.tensor_add(E_t, E_t, expP[:, tt, :])
            E = E_t

            # out[s, j] = sum_t expP[t, s] V[t, j];  l[s] = sum_p E[p, s]
            for sb in range(NSB):
                for su in range(SBLK // 128):
                    s0 = sb * SBLK + su * 128
                    opsum = psbo.tile([128, D], F32, tag="opsum")
                    lpsum = psbl.tile([128, 2], F32, tag="lpsum")
                    nc.tensor.matmul(
                        lpsum, E[:, s0 : s0 + 128], ones_t, start=True, stop=True
                    )
                    for tt in range(TT):
                        lhsT = expP[:, tt, s0 : s0 + 128]
                        nc.tensor.matmul(
                            opsum[:, 0:512],
                            lhsT,
                            V_sb[:, tt, 0:512],
                            start=(tt == 0),
                            stop=(tt == TT - 1),
                        )
                        nc.tensor.matmul(
                            opsum[:, 512:1024],
                            lhsT,
                            V_sb[:, tt, 512:1024],
                            start=(tt == 0),
                            stop=(tt == TT - 1),
                        )
                    recip = pbm.tile([128, 1], F32, tag="recip")
                    nc.vector.reciprocal(recip, lpsum[:, 0:1])
                    o_sb = pbo.tile([128, D], F32, tag="o_sb")
                    nc.vector.tensor_scalar_mul(o_sb, in0=opsum, scalar1=recip)
                    nc.sync.dma_start(out_r[sb * (SBLK // 128) + su], o_sb)


def build_nc():
    global _CACHED_NC
    if _CACHED_NC is not None:
        return _CACHED_NC
    import concourse.tile as tile
    from concourse import bacc, mybir

    F32 = mybir.dt.float32
    F32R = mybir.dt.float32r
    nc = bacc.Bacc("TRN2", target_bir_lowering=False, debug=False, num_devices=8)
    xt = nc.dram_tensor("xt", [D, HALF], F32R, kind="ExternalInput").ap()
    wq = nc.dram_tensor("wq", [128, DC, D], F32R, kind="ExternalInput").ap()
    wk = nc.dram_tensor("wk", [128, DC, D], F32R, kind="ExternalInput").ap()
    wv = nc.dram_tensor("wv", [128, DC, D], F32R, kind="ExternalInput").ap()
    bq = nc.dram_tensor("bq", [128, DC], F32, kind="ExternalInput").ap()
    bk = nc.dram_tensor("bk", [128, DC], F32, kind="ExternalInput").ap()
    bv = nc.dram_tensor("bv", [D], F32, kind="ExternalInput").ap()
    out = nc.dram_tensor("out", [HALF, D], F32, kind="ExternalOutput").ap()

    with tile.TileContext(nc) as tc:
        build_kernel(tc, xt, wq, wk, wv, bq, bk, bv, out)
    nc.compile()
    _CACHED_NC = nc
    return nc


def _shard_inputs(x, Wq, bq, Wk, bk, Wv, bv):
    """Host-side prep: per-core permuted x^T + relaid-out weights/biases."""
    wq_r = np.ascontiguousarray(Wq.reshape(DC, 128, D).transpose(1, 0, 2))
    wk_r = np.ascontiguousarray(Wk.reshape(DC, 128, D).transpose(1, 0, 2))
    wv_r = np.ascontiguousarray(Wv.reshape(DC, 128, D).transpose(1, 0, 2))
    bq_r = np.ascontiguousarray(bq.reshape(DC, 128).T)
    bk_r = np.ascontiguousarray(bk.reshape(DC, 128).T)
    bv_c = np.ascontiguousarray(bv)

    in_maps = []
    for c in range(NC):
        b, h = divmod(c, 2)
        xt = np.ascontiguousarray(x[b, h * HALF : (h + 1) * HALF].T)  # [D, HALF]
        in_maps.append(
            {
                "xt": xt,
                "wq": wq_r,
                "wk": wk_r,
                "wv": wv_r,
                "bq": bq_r,
                "bk": bk_r,
                "bv": bv_c,
            }
        )
    return in_maps


def kernel(x, Wq, bq, Wk, bk, Wv, bv):
    global LAST_RESULT
    _ensure_axon_ntff_hook()
    from concourse import bass_utils

    x = np.asarray(x, dtype=np.float32)
    args = [np.asarray(a, dtype=np.float32) for a in (Wq, bq, Wk, bk, Wv, bv)]
    nc = build_nc()
    in_maps = _shard_inputs(x, *args)
    res = bass_utils.run_bass_kernel_spmd(nc, in_maps, core_ids=list(range(NC)))
    LAST_RESULT = res
    out = np.empty((B, S, D), dtype=np.float32)
    for c in range(NC):
        b, h = divmod(c, 2)
        out[b, h * HALF : (h + 1) * HALF, :] = res.results[c]["out"]
    return out


if __name__ == "__main__":
    rng = np.random.default_rng(0)
    init = 1.0 / 32.0
    x = rng.standard_normal((B, S, D), dtype=np.float32)
    mk = lambda *s: rng.uniform(-init, init, s).astype(np.float32)
    o = kernel(x, mk(D, D), mk(D), mk(D, D), mk(D), mk(D, D), mk(D))
    print("out", o.shape, o.dtype, float(np.abs(o).max()))


# revision 10
# speedup vs baseline: 1.2996x; 1.2996x over previous
"""Single-head self-attention (B=4, S=2048, D=1024) on 8 trn2 NeuronCores.

Sharding: core c -> (batch b = c//2, query half h = c%2). Each core gets a
permuted x^T for its batch (own seq-half first; softmax is invariant to key
permutation), computes Q for its 1024 queries and K/V for all 2048 keys,
then attention. Output rows are the core's own queries in original order, so
the host gather is a pure concatenation.

Device layout (per core):
  xt (input)  : [1024_d, 2048_t] fp32   (x_perm.T, host-prepared)
  Q^T         : [128_dp, 8_dc, 512_s]   per s-block, SBUF
  K^T         : [128_jp, 8_jc, 2048_t]  spilled to HBM scratch, streamed back
  V           : [128_tp, 16_tc, 1024_j] SBUF-resident
  scores^T    : [128_t, 512_s] PSUM -> exp -> SBUF (keys on partitions, so
                attn@V needs no transposes; softmax sum over keys is an
                extra N=1 matmul vs a ones vector sharing the stationary
                operand; max-subtraction skipped: scores ~ N(0, 0.33))
All matmuls fp32r (fp22 mantissa, full PE rate at N>=256, fp32 accumulate).
"""

import os
import sys
import types

import numpy as np

B, S, D = 4, 2048, 1024
HALF = S // 2  # 1024 queries per core
SCALE = 1.0 / 32.0  # 1/sqrt(D)
NC = 8
DC = D // 128  # 8 d-chunks
TT = S // 128  # 16 key tiles
SBLK = 512  # queries per s-block
NSB = HALF // SBLK  # 2 s-blocks

_CACHED_NC = None
LAST_RESULT = None  # BassKernelResults of the most recent run (for test.py)


def _ensure_axon_ntff_hook():
    """bass_utils' trace path needs antenv.axon_hooks; this image's antenv
    lacks it. Install a shim backed by trn_agent_boot's ctypes hook so
    BASS_TRACE=1 profiling works. No-op if already present/unavailable."""
    try:
        import antenv.axon_hooks  # noqa: F401

        return
    except ImportError:
        pass
    try:
        from trn_agent_boot.trn_boot import _ntff_profile_via_ctypes

        hook = _ntff_profile_via_ctypes("/opt/axon/libaxon_pjrt.so")
    except Exception:
        hook = None
    mod = types.ModuleType("antenv.axon_hooks")
    mod.get_axon_ntff_profile_hook = lambda: hook
    mod.set_axon_ntff_profile_hook = lambda h: None
    sys.modules["antenv.axon_hooks"] = mod


def build_kernel(tc, xt, wq, wk, wv, bq, bk, bv, out):
    import concourse.bass as bass
    from concourse import mybir

    nc = tc.nc
    F32 = mybir.dt.float32
    F32R = mybir.dt.float32r
    Identity = mybir.ActivationFunctionType.Identity
    Exp = mybir.ActivationFunctionType.Exp

    xt_r = xt.rearrange("(c p) t -> p c t", p=128)  # [128, 8, 2048]
    out_r = out.rearrange("(su p) j -> su p j", p=128)  # [8, 128, 1024]

    with (
        tc.tile_pool(name="persist", bufs=1) as persist,
        tc.tile_pool(name="dram", bufs=1, space="DRAM") as dram,
    ):
        V_sb = persist.tile([128, TT, D], F32R)
        qT = persist.tile([128, DC, HALF], F32R)
        bv_bc = persist.tile([128, D], F32)
        bv_bcast_ap = bass.AP(
            tensor=bv.tensor, offset=bv.offset, ap=[[0, 128]] + list(bv.ap)
        )
        nc.scalar.dma_start(bv_bc, bv_bcast_ap)
        bq_sb = persist.tile([128, DC], F32)
        nc.scalar.dma_start(bq_sb, bq)
        bk_sb = persist.tile([128, DC], F32)
        nc.scalar.dma_start(bk_sb, bk)
        ones_f = persist.tile([128, 2], F32)
        nc.vector.memset(ones_f, 1.0)
        ones_t = persist.tile([128, 2], F32R)
        nc.vector.tensor_copy(ones_t, ones_f)

        kT_dram = dram.tile([128, DC, S], F32R)

        # ------------- Phase A: Q^T, K^T (-> HBM), V (resident) -----------
        with (
            tc.tile_pool(name="pa", bufs=1) as pa,
            tc.tile_pool(name="pa_w", bufs=3) as paw,
            tc.tile_pool(name="pa_st", bufs=2) as past,
            tc.tile_pool(name="psa", bufs=2, space="PSUM") as psa,
        ):
            # x^T loaded in per-chunk DMAs so matmuls start as data lands;
            # weights go on the Scalar engine's DMA queue so they are not
            # stuck behind the 8MB x^T stream on the Sync queue.
            xT = pa.tile([128, DC, S], F32R)
            for c in range(DC):
                nc.sync.dma_start(xT[:, c, :], xt_r[:, c, :])

            # Q^T[dq, s] = sum_d Wq[d, dq] xT[d, s]  (+bq fused on copy-out)
            for qc in range(DC):
                wq_t = paw.tile([128, DC, 128], F32R, tag="w_t")
                nc.scalar.dma_start(wq_t, wq[:, :, qc * 128 : (qc + 1) * 128])
                for sblk in range(NSB):
                    qpsum = psa.tile([128, SBLK], F32, tag="qpsum")
                    for c in range(DC):
                        nc.tensor.matmul(
                            qpsum,
                            wq_t[:, c, :],
                            xT[:, c, sblk * SBLK : (sblk + 1) * SBLK],
                            start=(c == 0),
                            stop=(c == DC - 1),
                        )
                    nc.scalar.activation(
                        qT[:, qc, sblk * SBLK : (sblk + 1) * SBLK],
                        qpsum,
                        Identity,
                        bias=bq_sb[:, qc : qc + 1],
                    )

            # K^T[j, t] (+bk) -> HBM scratch
            def k_group(jt):
                wk_t = paw.tile([128, DC, 128], F32R, tag="w_t")
                nc.scalar.dma_start(wk_t, wk[:, :, jt * 128 : (jt + 1) * 128])
                for tb in range(S // 512):
                    kpsum = psa.tile([128, 512], F32, tag="kpsum")
                    for c in range(DC):
                        nc.tensor.matmul(
                            kpsum,
                            wk_t[:, c, :],
                            xT[:, c, tb * 512 : (tb + 1) * 512],
                            start=(c == 0),
                            stop=(c == DC - 1),
                        )
                    kstage = past.tile([128, 512], F32R, tag="kstage")
                    nc.scalar.activation(
                        kstage, kpsum, Identity, bias=bk_sb[:, jt : jt + 1]
                    )
                    nc.sync.dma_start(
                        kT_dram[:, jt, tb * 512 : (tb + 1) * 512], kstage
                    )

            # V[t, j] (+bv), one j-half at a time (wv streamed)
            def v_half(jb):
                wv_h = pa.tile([128, DC, 512], F32R, tag="wv_h")
                nc.scalar.dma_start(wv_h, wv[:, :, jb * 512 : (jb + 1) * 512])
                for tt in range(TT):
                    vpsum = psa.tile([128, 512], F32, tag="vpsum")
                    for c in range(DC):
                        nc.tensor.matmul(
                            vpsum,
                            xT[:, c, tt * 128 : (tt + 1) * 128],
                            wv_h[:, c, :],
                            start=(c == 0),
                            stop=(c == DC - 1),
                        )
                    nc.vector.tensor_add(
                        V_sb[:, tt, jb * 512 : (jb + 1) * 512],
                        vpsum,
                        bv_bc[:, jb * 512 : (jb + 1) * 512],
                    )

            # interleave so wv half-reloads hide under K-tile matmuls
            for jt in range(4):
                k_group(jt)
            v_half(0)
            for jt in range(4, DC):
                k_group(jt)
            v_half(1)

        # ------------- Phase B: scores^T, softmax, out --------------------
        with (
            tc.tile_pool(name="pb_kt", bufs=2) as pbkt,
            tc.tile_pool(name="pb_p", bufs=1) as pbp,
            tc.tile_pool(name="pb_o", bufs=2) as pbo,
            tc.tile_pool(name="pb_m", bufs=2) as pbm,
            tc.tile_pool(name="psb_s", bufs=2, space="PSUM") as psbs,
            tc.tile_pool(name="psb_o", bufs=2, space="PSUM") as psbo,
            tc.tile_pool(name="psb_l", bufs=2, space="PSUM") as psbl,
        ):
            # scores^T + exp for both s-blocks per K^T tile (K^T read once)
            expP = pbp.tile([128, TT, HALF], F32R, tag="expP")
            for tt in range(TT):
                kt_t = pbkt.tile([128, DC, 128], F32R, tag="kt_t")
                nc.sync.dma_start(kt_t, kT_dram[:, :, tt * 128 : (tt + 1) * 128])
                for sb in range(NSB):
                    spsum = psbs.tile([128, SBLK], F32, tag="spsum")
                    for jc in range(DC):
                        nc.tensor.matmul(
                            spsum,
                            kt_t[:, jc, :],
                            qT[:, jc, sb * SBLK : (sb + 1) * SBLK],
                            start=(jc == 0),
                            stop=(jc == DC - 1),
                        )
                    nc.scalar.activation(
                        expP[:, tt, sb * SBLK : (sb + 1) * SBLK],
                        spsum,
                        Exp,
                        scale=SCALE,
                    )

            # E[p, s] = sum_tt expP[p, tt, s] via a DVE accumulate chain;
            # the softmax sum l[s] then needs only one matmul per query tile.
            E_t = pbp.tile([128, HALF], F32R, tag="E_t")
            nc.vector.tensor_add(E_t, expP[:, 0, :], expP[:, 1, :])
            for tt in range(2, TT):
                nc.vector.tensor_add(E_t, E_t, expP[:, tt, :])
            E = E_t

            # out[s, j] = sum_t expP[t, s] V[t, j];  l[s] = sum_p E[p, s]
            for sb in range(NSB):
                for su in range(SBLK // 128):
                    s0 = sb * SBLK + su * 128
                    opsum = psbo.tile([128, D], F32, tag="opsum")
                    lpsum = psbl.tile([128, 2], F32, tag="lpsum")
                    nc.tensor.matmul(
                        lpsum, E[:, s0 : s0 + 128], ones_t, start=True, stop=True
                    )
                    for tt in range(TT):
                        lhsT = expP[:, tt, s0 : s0 + 128]
                        nc.tensor.matmul(
                            opsum[:, 0:512],
                            lhsT,
                            V_sb[:, tt, 0:512],
                            start=(tt == 0),
                            stop=(tt == TT - 1),
                        )
                        nc.tensor.matmul(
                            opsum[:, 512:1024],
                            lhsT,
                            V_sb[:, tt, 512:1024],
                            start=(tt == 0),
                            stop=(tt == TT - 1),
                        )
                    recip = pbm.tile([128, 1], F32, tag="recip")
                    nc.vector.reciprocal(recip, lpsum[:, 0:1])
                    o_sb = pbo.tile([128, D], F32, tag="o_sb")
                    nc.vector.tensor_scalar_mul(o_sb, in0=opsum, scalar1=recip)
                    nc.sync.dma_start(out_r[sb * (SBLK // 128) + su], o_sb)


def build_nc():
    global _CACHED_NC
    if _CACHED_NC is not None:
        return _CACHED_NC
    import concourse.tile as tile
    from concourse import bacc, mybir

    F32 = mybir.dt.float32
    F32R = mybir.dt.float32r
    nc = bacc.Bacc("TRN2", target_bir_lowering=False, debug=False)
    xt = nc.dram_tensor("xt", [D, S], F32R, kind="ExternalInput").ap()
    wq = nc.dram_tensor("wq", [128, DC, D], F32R, kind="ExternalInput").ap()
    wk = nc.dram_tensor("wk", [128, DC, D], F32R, kind="ExternalInput").ap()
    wv = nc.dram_tensor("wv", [128, DC, D], F32R, kind="ExternalInput").ap()
    bq = nc.dram_tensor("bq", [128, DC], F32, kind="ExternalInput").ap()
    bk = nc.dram_tensor("bk", [128, DC], F32, kind="ExternalInput").ap()
    bv = nc.dram_tensor("bv", [D], F32, kind="ExternalInput").ap()
    out = nc.dram_tensor("out", [HALF, D], F32, kind="ExternalOutput").ap()

    with tile.TileContext(nc) as tc:
        build_kernel(tc, xt, wq, wk, wv, bq, bk, bv, out)
    nc.compile()
    _CACHED_NC = nc
    return nc


def _shard_inputs(x, Wq, bq, Wk, bk, Wv, bv):
    """Host-side prep: per-core permuted x^T + relaid-out weights/biases."""
    wq_r = np.ascontiguousarray(Wq.reshape(DC, 128, D).transpose(1, 0, 2))
    wk_r = np.ascontiguousarray(Wk.reshape(DC, 128, D).transpose(1, 0, 2))
    wv_r = np.ascontiguousarray(Wv.reshape(DC, 128, D).transpose(1, 0, 2))
    bq_r = np.ascontiguousarray(bq.reshape(DC, 128).T)
    bk_r = np.ascontiguousarray(bk.reshape(DC, 128).T)
    bv_c = np.ascontiguousarray(bv)

    in_maps = []
    for c in range(NC):
        b, h = divmod(c, 2)
        xb = x[b]
        if h:
            xb = np.concatenate([xb[HALF:], xb[:HALF]], axis=0)
        xt = np.ascontiguousarray(xb.T)  # [D, S], own queries first
        in_maps.append(
            {
                "xt": xt,
                "wq": wq_r,
                "wk": wk_r,
                "wv": wv_r,
                "bq": bq_r,
                "bk": bk_r,
                "bv": bv_c,
            }
        )
    return in_maps


def kernel(x, Wq, bq, Wk, bk, Wv, bv):
    global LAST_RESULT
    _ensure_axon_ntff_hook()
    from concourse import bass_utils

    x = np.asarray(x, dtype=np.float32)
    args = [np.asarray(a, dtype=np.float32) for a in (Wq, bq, Wk, bk, Wv, bv)]
    nc = build_nc()
    in_maps = _shard_inputs(x, *args)
    res = bass_utils.run_bass_kernel_spmd(nc, in_maps, core_ids=list(range(NC)))
    LAST_RESULT = res
    out = np.empty((B, S, D), dtype=np.float32)
    for c in range(NC):
        b, h = divmod(c, 2)
        out[b, h * HALF : (h + 1) * HALF, :] = res.results[c]["out"]
    return out


if __name__ == "__main__":
    rng = np.random.default_rng(0)
    init = 1.0 / 32.0
    x = rng.standard_normal((B, S, D), dtype=np.float32)
    mk = lambda *s: rng.uniform(-init, init, s).astype(np.float32)
    o = kernel(x, mk(D, D), mk(D), mk(D, D), mk(D), mk(D, D), mk(D))
    print("out", o.shape, o.dtype, float(np.abs(o).max()))


# revision 11
# speedup vs baseline: 1.3586x; 1.0454x over previous
"""Single-head self-attention (B=4, S=2048, D=1024) on 8 trn2 NeuronCores.

Sharding: core c -> (batch b = c//2, query half h = c%2). Each core gets a
permuted x^T for its batch (own seq-half first; softmax is invariant to key
permutation), computes Q for its 1024 queries and K/V for all 2048 keys,
then attention. Output rows are the core's own queries in original order, so
the host gather is a pure concatenation.

Device layout (per core):
  xt (input)  : [1024_d, 2048_t] fp32   (x_perm.T, host-prepared)
  Q^T         : [128_dp, 8_dc, 512_s]   per s-block, SBUF
  K^T         : [128_jp, 8_jc, 2048_t]  spilled to HBM scratch, streamed back
  V           : [128_tp, 16_tc, 1024_j] SBUF-resident
  scores^T    : [128_t, 512_s] PSUM -> exp -> SBUF (keys on partitions, so
                attn@V needs no transposes; softmax sum over keys is an
                extra N=1 matmul vs a ones vector sharing the stationary
                operand; max-subtraction skipped: scores ~ N(0, 0.33))
All matmuls fp32r (fp22 mantissa, full PE rate at N>=256, fp32 accumulate).
"""

import os
import sys
import types

import numpy as np

B, S, D = 4, 2048, 1024
HALF = S // 2  # 1024 queries per core
SCALE = 1.0 / 32.0  # 1/sqrt(D)
NC = 8
DC = D // 128  # 8 d-chunks
TT = S // 128  # 16 key tiles
SBLK = 512  # queries per s-block
NSB = HALF // SBLK  # 2 s-blocks

_CACHED_NC = None
LAST_RESULT = None  # BassKernelResults of the most recent run (for test.py)


def _ensure_axon_ntff_hook():
    """bass_utils' trace path needs antenv.axon_hooks; this image's antenv
    lacks it. Install a shim backed by trn_agent_boot's ctypes hook so
    BASS_TRACE=1 profiling works. No-op if already present/unavailable."""
    try:
        import antenv.axon_hooks  # noqa: F401

        return
    except ImportError:
        pass
    try:
        from trn_agent_boot.trn_boot import _ntff_profile_via_ctypes

        hook = _ntff_profile_via_ctypes("/opt/axon/libaxon_pjrt.so")
    except Exception:
        hook = None
    mod = types.ModuleType("antenv.axon_hooks")
    mod.get_axon_ntff_profile_hook = lambda: hook
    mod.set_axon_ntff_profile_hook = lambda h: None
    sys.modules["antenv.axon_hooks"] = mod


def build_kernel(tc, xt, wq, wk, wv, bq, bv, out):
    import concourse.bass as bass
    from concourse import mybir

    nc = tc.nc
    F32 = mybir.dt.float32
    F32R = mybir.dt.float32r
    Identity = mybir.ActivationFunctionType.Identity
    Copy = mybir.ActivationFunctionType.Copy
    Exp = mybir.ActivationFunctionType.Exp

    xt_r = xt.rearrange("(c p) t -> p c t", p=128)  # [128, 8, 2048]
    out_r = out.rearrange("(su p) j -> su p j", p=128)  # [8, 128, 1024]

    with (
        tc.tile_pool(name="persist", bufs=1) as persist,
        tc.tile_pool(name="pkt", bufs=2) as pkt,
        tc.tile_pool(name="dram", bufs=1, space="DRAM") as dram,
    ):
        V_sb = persist.tile([128, TT, D], F32R)
        qT = persist.tile([128, DC, HALF], F32R)
        bv_bc = persist.tile([128, D], F32)
        bv_bcast_ap = bass.AP(
            tensor=bv.tensor, offset=bv.offset, ap=[[0, 128]] + list(bv.ap)
        )
        nc.scalar.dma_start(bv_bc, bv_bcast_ap)
        bq_sb = persist.tile([128, DC], F32)
        nc.scalar.dma_start(bq_sb, bq)
        ones_f = persist.tile([128, 2], F32)
        nc.vector.memset(ones_f, 1.0)
        ones_t = persist.tile([128, 2], F32R)
        nc.vector.tensor_copy(ones_t, ones_f)

        kT_dram = dram.tile([128, DC, S], F32R)

        # ------------- Phase A: K^T (-> HBM), Q^T, V (resident) -----------
        # NOTE: the K bias is dropped entirely -- q.(k+bk) = q.k + q.bk and
        # the q.bk term is constant across keys, so softmax cancels it.
        with (
            tc.tile_pool(name="pa", bufs=1) as pa,
            tc.tile_pool(name="pa_wk", bufs=2) as pawk,
            tc.tile_pool(name="pa_wq", bufs=2) as pawq,
            tc.tile_pool(name="pa_st", bufs=1) as past,
            tc.tile_pool(name="psa", bufs=2, space="PSUM") as psa,
        ):
            # x^T loaded t-block-major so the first K-tile groups (which
            # need every d-chunk of one 512-column block) unblock earliest.
            xT = pa.tile([128, DC, S], F32R)
            for tb in range(S // 512):
                for c in range(DC):
                    nc.sync.dma_start(
                        xT[:, c, tb * 512 : (tb + 1) * 512],
                        xt_r[:, c, tb * 512 : (tb + 1) * 512],
                    )

            def k_group(jt, tb, wk_t):
                kpsum = psa.tile([128, 512], F32, tag="kpsum")
                for c in range(DC):
                    nc.tensor.matmul(
                        kpsum,
                        wk_t[:, c, :],
                        xT[:, c, tb * 512 : (tb + 1) * 512],
                        start=(c == 0),
                        stop=(c == DC - 1),
                    )
                kstage = past.tile([128, 512], F32R, tag="kstage")
                nc.scalar.activation(kstage, kpsum, Copy)
                nc.sync.dma_start(
                    kT_dram[:, jt, tb * 512 : (tb + 1) * 512], kstage
                )

            # K^T pass 1: all j-tiles of t-block 0 (earliest-ready work)
            for jt in range(DC):
                wk_t = pawk.tile([128, DC, 128], F32R, tag="wk_t")
                nc.scalar.dma_start(wk_t, wk[:, :, jt * 128 : (jt + 1) * 128])
                k_group(jt, 0, wk_t)
            # K^T pass 2: remaining t-blocks, j-tile major (wk re-streamed)
            for jt in range(DC):
                wk_t = pawk.tile([128, DC, 128], F32R, tag="wk_t")
                nc.scalar.dma_start(wk_t, wk[:, :, jt * 128 : (jt + 1) * 128])
                for tb in range(1, S // 512):
                    k_group(jt, tb, wk_t)

            # Q^T[dq, s] = sum_d Wq[d, dq] xT[d, s]  (+bq fused on copy-out)
            for qc in range(DC):
                wq_t = pawq.tile([128, DC, 128], F32R, tag="wq_t")
                nc.scalar.dma_start(wq_t, wq[:, :, qc * 128 : (qc + 1) * 128])
                for sblk in range(NSB):
                    qpsum = psa.tile([128, SBLK], F32, tag="qpsum")
                    for c in range(DC):
                        nc.tensor.matmul(
                            qpsum,
                            wq_t[:, c, :],
                            xT[:, c, sblk * SBLK : (sblk + 1) * SBLK],
                            start=(c == 0),
                            stop=(c == DC - 1),
                        )
                    nc.scalar.activation(
                        qT[:, qc, sblk * SBLK : (sblk + 1) * SBLK],
                        qpsum,
                        Identity,
                        bias=bq_sb[:, qc : qc + 1],
                    )

            # V[t, j] (+bv), one j-half at a time (wv streamed)
            for jb in range(2):
                wv_h = pa.tile([128, DC, 512], F32R, tag="wv_h")
                nc.scalar.dma_start(wv_h, wv[:, :, jb * 512 : (jb + 1) * 512])
                for tt in range(TT):
                    vpsum = psa.tile([128, 512], F32, tag="vpsum")
                    for c in range(DC):
                        nc.tensor.matmul(
                            vpsum,
                            xT[:, c, tt * 128 : (tt + 1) * 128],
                            wv_h[:, c, :],
                            start=(c == 0),
                            stop=(c == DC - 1),
                        )
                    nc.vector.tensor_add(
                        V_sb[:, tt, jb * 512 : (jb + 1) * 512],
                        vpsum,
                        bv_bc[:, jb * 512 : (jb + 1) * 512],
                    )

        # ------------- Phase B: scores^T, softmax, out --------------------
        with (
            tc.tile_pool(name="pb_p", bufs=1) as pbp,
            tc.tile_pool(name="pb_o", bufs=2) as pbo,
            tc.tile_pool(name="pb_m", bufs=2) as pbm,
            tc.tile_pool(name="psb_s", bufs=2, space="PSUM") as psbs,
            tc.tile_pool(name="psb_o", bufs=2, space="PSUM") as psbo,
            tc.tile_pool(name="psb_l", bufs=2, space="PSUM") as psbl,
        ):
            # scores^T + exp for both s-blocks per K^T tile (K^T read once).
            # E[p, s] = sum_tt expP[p, tt, s] accumulates on DVE as exps land
            # so the softmax sum l[s] needs only one matmul per query tile.
            expP = pbp.tile([128, TT, HALF], F32R, tag="expP")
            E_t = pbp.tile([128, HALF], F32R, tag="E_t")
            for tt in range(TT):
                kt_t = pkt.tile([128, DC, 128], F32R, tag="kt_t")
                nc.sync.dma_start(kt_t, kT_dram[:, :, tt * 128 : (tt + 1) * 128])
                for sb in range(NSB):
                    spsum = psbs.tile([128, SBLK], F32, tag="spsum")
                    for jc in range(DC):
                        nc.tensor.matmul(
                            spsum,
                            kt_t[:, jc, :],
                            qT[:, jc, sb * SBLK : (sb + 1) * SBLK],
                            start=(jc == 0),
                            stop=(jc == DC - 1),
                        )
                    nc.scalar.activation(
                        expP[:, tt, sb * SBLK : (sb + 1) * SBLK],
                        spsum,
                        Exp,
                        scale=SCALE,
                    )
                if tt == 1:
                    nc.vector.tensor_add(E_t, expP[:, 0, :], expP[:, 1, :])
                elif tt > 1:
                    nc.vector.tensor_add(E_t, E_t, expP[:, tt, :])

            # out[s, j] = sum_t expP[t, s] V[t, j];  l[s] = sum_p E[p, s]
            for sb in range(NSB):
                for su in range(SBLK // 128):
                    s0 = sb * SBLK + su * 128
                    opsum = psbo.tile([128, D], F32, tag="opsum")
                    lpsum = psbl.tile([128, 2], F32, tag="lpsum")
                    for tt in range(TT):
                        lhsT = expP[:, tt, s0 : s0 + 128]
                        nc.tensor.matmul(
                            opsum[:, 0:512],
                            lhsT,
                            V_sb[:, tt, 0:512],
                            start=(tt == 0),
                            stop=(tt == TT - 1),
                        )
                        nc.tensor.matmul(
                            opsum[:, 512:1024],
                            lhsT,
                            V_sb[:, tt, 512:1024],
                            start=(tt == 0),
                            stop=(tt == TT - 1),
                        )
                    nc.tensor.matmul(
                        lpsum, E_t[:, s0 : s0 + 128], ones_t, start=True, stop=True
                    )
                    recip = pbm.tile([128, 1], F32, tag="recip")
                    nc.vector.reciprocal(recip, lpsum[:, 0:1])
                    o_sb = pbo.tile([128, D], F32, tag="o_sb")
                    nc.vector.tensor_scalar_mul(o_sb, in0=opsum, scalar1=recip)
                    nc.sync.dma_start(out_r[sb * (SBLK // 128) + su], o_sb)


def build_nc():
    global _CACHED_NC
    if _CACHED_NC is not None:
        return _CACHED_NC
    import concourse.tile as tile
    from concourse import bacc, mybir

    F32 = mybir.dt.float32
    F32R = mybir.dt.float32r
    nc = bacc.Bacc("TRN2", target_bir_lowering=False, debug=False)
    xt = nc.dram_tensor("xt", [D, S], F32R, kind="ExternalInput").ap()
    wq = nc.dram_tensor("wq", [128, DC, D], F32R, kind="ExternalInput").ap()
    wk = nc.dram_tensor("wk", [128, DC, D], F32R, kind="ExternalInput").ap()
    wv = nc.dram_tensor("wv", [128, DC, D], F32R, kind="ExternalInput").ap()
    bq = nc.dram_tensor("bq", [128, DC], F32, kind="ExternalInput").ap()
    bv = nc.dram_tensor("bv", [D], F32, kind="ExternalInput").ap()
    out = nc.dram_tensor("out", [HALF, D], F32, kind="ExternalOutput").ap()

    with tile.TileContext(nc) as tc:
        build_kernel(tc, xt, wq, wk, wv, bq, bv, out)
    nc.compile()
    _CACHED_NC = nc
    return nc


def _shard_inputs(x, Wq, bq, Wk, bk, Wv, bv):
    """Host-side prep: per-core permuted x^T + relaid-out weights/biases."""
    wq_r = np.ascontiguousarray(Wq.reshape(DC, 128, D).transpose(1, 0, 2))
    wk_r = np.ascontiguousarray(Wk.reshape(DC, 128, D).transpose(1, 0, 2))
    wv_r = np.ascontiguousarray(Wv.reshape(DC, 128, D).transpose(1, 0, 2))
    bq_r = np.ascontiguousarray(bq.reshape(DC, 128).T)
    bv_c = np.ascontiguousarray(bv)

    in_maps = []
    for c in range(NC):
        b, h = divmod(c, 2)
        xb = x[b]
        if h:
            xb = np.concatenate([xb[HALF:], xb[:HALF]], axis=0)
        xt = np.ascontiguousarray(xb.T)  # [D, S], own queries first
        in_maps.append(
            {
                "xt": xt,
                "wq": wq_r,
                "wk": wk_r,
                "wv": wv_r,
                "bq": bq_r,
                "bv": bv_c,
            }
        )
    return in_maps


def kernel(x, Wq, bq, Wk, bk, Wv, bv):
    global LAST_RESULT
    _ensure_axon_ntff_hook()
    from concourse import bass_utils

    x = np.asarray(x, dtype=np.float32)
    args = [np.asarray(a, dtype=np.float32) for a in (Wq, bq, Wk, bk, Wv, bv)]
    nc = build_nc()
    in_maps = _shard_inputs(x, *args)
    res = bass_utils.run_bass_kernel_spmd(nc, in_maps, core_ids=list(range(NC)))
    LAST_RESULT = res
    out = np.empty((B, S, D), dtype=np.float32)
    for c in range(NC):
        b, h = divmod(c, 2)
        out[b, h * HALF : (h + 1) * HALF, :] = res.results[c]["out"]
    return out


if __name__ == "__main__":
    rng = np.random.default_rng(0)
    init = 1.0 / 32.0
    x = rng.standard_normal((B, S, D), dtype=np.float32)
    mk = lambda *s: rng.uniform(-init, init, s).astype(np.float32)
    o = kernel(x, mk(D, D), mk(D), mk(D, D), mk(D), mk(D, D), mk(D))
    print("out", o.shape, o.dtype, float(np.abs(o).max()))


# revision 13
# speedup vs baseline: 1.4403x; 1.0601x over previous
"""Single-head self-attention (B=4, S=2048, D=1024) on 8 trn2 NeuronCores.

Sharding: core c -> (batch b = c//2, query half h = c%2). Each core gets a
permuted x^T for its batch (own seq-half first; softmax is invariant to key
permutation), computes Q for its 1024 queries and K/V for all 2048 keys,
then attention. Output rows are the core's own queries in original order, so
the host gather is a pure concatenation.

Device layout (per core):
  xt (input)  : [1024_d, 2048_t] fp32   (x_perm.T, host-prepared)
  Q^T         : [128_dp, 8_dc, 512_s]   per s-block, SBUF
  K^T         : [128_jp, 8_jc, 2048_t]  spilled to HBM scratch, streamed back
  V           : [128_tp, 16_tc, 1024_j] SBUF-resident
  scores^T    : [128_t, 512_s] PSUM -> exp -> SBUF (keys on partitions, so
                attn@V needs no transposes; softmax sum over keys is an
                extra N=1 matmul vs a ones vector sharing the stationary
                operand; max-subtraction skipped: scores ~ N(0, 0.33))
All matmuls fp32r (fp22 mantissa, full PE rate at N>=256, fp32 accumulate).
"""

import os
import sys
import types

import numpy as np

B, S, D = 4, 2048, 1024
HALF = S // 2  # 1024 queries per core
SCALE = 1.0 / 32.0  # 1/sqrt(D)
NC = 8
DC = D // 128  # 8 d-chunks
TT = S // 128  # 16 key tiles
SBLK = 512  # queries per s-block
NSB = HALF // SBLK  # 2 s-blocks

_CACHED_NC = None
LAST_RESULT = None  # BassKernelResults of the most recent run (for test.py)


def _ensure_axon_ntff_hook():
    """bass_utils' trace path needs antenv.axon_hooks; this image's antenv
    lacks it. Install a shim backed by trn_agent_boot's ctypes hook so
    BASS_TRACE=1 profiling works. No-op if already present/unavailable."""
    try:
        import antenv.axon_hooks  # noqa: F401

        return
    except ImportError:
        pass
    try:
        from trn_agent_boot.trn_boot import _ntff_profile_via_ctypes

        hook = _ntff_profile_via_ctypes("/opt/axon/libaxon_pjrt.so")
    except Exception:
        hook = None
    mod = types.ModuleType("antenv.axon_hooks")
    mod.get_axon_ntff_profile_hook = lambda: hook
    mod.set_axon_ntff_profile_hook = lambda h: None
    sys.modules["antenv.axon_hooks"] = mod


def build_kernel(tc, xt, wq, wk, wv, bq, bv, out):
    """Per-core attention, no K^T materialization:
      Q^T = Wq^T-proj of own queries (+bq)          [128, DC, 1024_s]
      G   = Wk @ Q^T  (contraction over K's embed)  [128, DC, 1024_s]
      scores^T[t, s] = sum_d xT[d, t] G[d, s]  -- so only x^T (resident)
      and the small G are needed; the K bias cancels in softmax.
    """
    import concourse.bass as bass
    from concourse import mybir

    nc = tc.nc
    F32 = mybir.dt.float32
    F32R = mybir.dt.float32r
    Identity = mybir.ActivationFunctionType.Identity
    Copy = mybir.ActivationFunctionType.Copy
    Exp = mybir.ActivationFunctionType.Exp

    xt_r = xt.rearrange("(c p) t -> p c t", p=128)  # [128, 8, 2048]
    out_r = out.rearrange("(su p) j -> su p j", p=128)  # [8, 128, 1024]

    with tc.tile_pool(name="persist", bufs=1) as persist:
        xT = persist.tile([128, DC, S], F32R)
        V_sb = persist.tile([128, TT, D], F32R)
        G = persist.tile([128, DC, HALF], F32R)
        bv_bc = persist.tile([128, D], F32)
        bv_bcast_ap = bass.AP(
            tensor=bv.tensor, offset=bv.offset, ap=[[0, 128]] + list(bv.ap)
        )
        nc.scalar.dma_start(bv_bc, bv_bcast_ap)
        bq_sb = persist.tile([128, DC], F32)
        nc.scalar.dma_start(bq_sb, bq)
        ones_f = persist.tile([128, 2], F32)
        nc.vector.memset(ones_f, 1.0)
        ones_t = persist.tile([128, 2], F32R)
        nc.vector.tensor_copy(ones_t, ones_f)

        # x^T loaded t-block-major so early query/score groups unblock first
        for tb in range(S // 512):
            for c in range(DC):
                nc.sync.dma_start(
                    xT[:, c, tb * 512 : (tb + 1) * 512],
                    xt_r[:, c, tb * 512 : (tb + 1) * 512],
                )

        # ---- Phase A1: Q^T then G = Wk @ Q^T ----------------------------
        psa_cm = tc.tile_pool(name="psa", bufs=2, space="PSUM")
        psa = psa_cm.__enter__()
        with (
            tc.tile_pool(name="pa1", bufs=1) as pa1,
            tc.tile_pool(name="pa_w", bufs=2) as paw,
        ):
            qT = pa1.tile([128, DC, HALF], F32R)
            for qc in range(DC):
                wq_t = paw.tile([128, DC, 128], F32R, tag="w_t")
                nc.scalar.dma_start(wq_t, wq[:, :, qc * 128 : (qc + 1) * 128])
                for sblk in range(NSB):
                    qpsum = psa.tile([128, SBLK], F32, tag="qpsum")
                    for c in range(DC):
                        nc.tensor.matmul(
                            qpsum,
                            wq_t[:, c, :],
                            xT[:, c, sblk * SBLK : (sblk + 1) * SBLK],
                            start=(c == 0),
                            stop=(c == DC - 1),
                        )
                    nc.scalar.activation(
                        qT[:, qc, sblk * SBLK : (sblk + 1) * SBLK],
                        qpsum,
                        Identity,
                        bias=bq_sb[:, qc : qc + 1],
                    )

            # G[d, s] = sum_j Wk[d, j] qT[j, s]  (wk passed j-major = Wk.T)
            for gc in range(DC):
                wk_t = paw.tile([128, DC, 128], F32R, tag="w_t")
                nc.scalar.dma_start(wk_t, wk[:, :, gc * 128 : (gc + 1) * 128])
                for sblk in range(NSB):
                    gpsum = psa.tile([128, SBLK], F32, tag="gpsum")
                    for jc in range(DC):
                        nc.tensor.matmul(
                            gpsum,
                            wk_t[:, jc, :],
                            qT[:, jc, sblk * SBLK : (sblk + 1) * SBLK],
                            start=(jc == 0),
                            stop=(jc == DC - 1),
                        )
                    nc.scalar.activation(
                        G[:, gc, sblk * SBLK : (sblk + 1) * SBLK], gpsum, Copy
                    )

        # ---- Phase A2: V[t, j] (+bv), wv streamed in j-halves ------------
        with tc.tile_pool(name="pa2", bufs=1) as pa2:
            for jb in range(2):
                wv_h = pa2.tile([128, DC, 512], F32R, tag="wv_h")
                nc.scalar.dma_start(wv_h, wv[:, :, jb * 512 : (jb + 1) * 512])
                for tt in range(TT):
                    vpsum = psa.tile([128, 512], F32, tag="vpsum")
                    for c in range(DC):
                        nc.tensor.matmul(
                            vpsum,
                            xT[:, c, tt * 128 : (tt + 1) * 128],
                            wv_h[:, c, :],
                            start=(c == 0),
                            stop=(c == DC - 1),
                        )
                    nc.vector.tensor_add(
                        V_sb[:, tt, jb * 512 : (jb + 1) * 512],
                        vpsum,
                        bv_bc[:, jb * 512 : (jb + 1) * 512],
                    )
        psa_cm.__exit__(None, None, None)

        # ---- Phase B: scores^T, softmax, out, one 512-query block at a time
        with (
            tc.tile_pool(name="pb_p", bufs=1) as pbp,
            tc.tile_pool(name="pb_o", bufs=2) as pbo,
            tc.tile_pool(name="pb_m", bufs=2) as pbm,
            tc.tile_pool(name="psb_s", bufs=2, space="PSUM") as psbs,
            tc.tile_pool(name="psb_o", bufs=2, space="PSUM") as psbo,
            tc.tile_pool(name="psb_l", bufs=2, space="PSUM") as psbl,
        ):
            for sb in range(NSB):
                # scores^T[t, s] = sum_d xT[d, t] G[d, s]; exp fused w/ 1/32.
                # E[p, s] accumulates exp sums on DVE as the tiles land.
                expP = pbp.tile([128, TT, SBLK], F32R, tag="expP")
                E_t = pbp.tile([128, SBLK], F32R, tag="E_t", bufs=1)
                for tt in range(TT):
                    spsum = psbs.tile([128, SBLK], F32, tag="spsum")
                    for c in range(DC):
                        nc.tensor.matmul(
                            spsum,
                            xT[:, c, tt * 128 : (tt + 1) * 128],
                            G[:, c, sb * SBLK : (sb + 1) * SBLK],
                            start=(c == 0),
                            stop=(c == DC - 1),
                        )
                    nc.scalar.activation(expP[:, tt, :], spsum, Exp, scale=SCALE)
                    if tt == 1:
                        nc.vector.tensor_add(E_t, expP[:, 0, :], expP[:, 1, :])
                    elif tt > 1:
                        nc.vector.tensor_add(E_t, E_t, expP[:, tt, :])

                # out[s, j] = sum_t expP[t, s] V[t, j];  l[s] = sum_p E[p, s]
                for su in range(SBLK // 128):
                    s0 = su * 128
                    opsum = psbo.tile([128, D], F32, tag="opsum")
                    lpsum = psbl.tile([128, 2], F32, tag="lpsum")
                    for tt in range(TT):
                        lhsT = expP[:, tt, s0 : s0 + 128]
                        nc.tensor.matmul(
                            opsum[:, 0:512],
                            lhsT,
                            V_sb[:, tt, 0:512],
                            start=(tt == 0),
                            stop=(tt == TT - 1),
                        )
                        nc.tensor.matmul(
                            opsum[:, 512:1024],
                            lhsT,
                            V_sb[:, tt, 512:1024],
                            start=(tt == 0),
                            stop=(tt == TT - 1),
                        )
                    nc.tensor.matmul(
                        lpsum, E_t[:, s0 : s0 + 128], ones_t, start=True, stop=True
                    )
                    recip = pbm.tile([128, 1], F32, tag="recip")
                    nc.vector.reciprocal(recip, lpsum[:, 0:1])
                    o_sb = pbo.tile([128, D], F32, tag="o_sb")
                    nc.vector.tensor_scalar_mul(o_sb, in0=opsum, scalar1=recip)
                    nc.sync.dma_start(out_r[sb * (SBLK // 128) + su], o_sb)


def build_nc():
    global _CACHED_NC
    if _CACHED_NC is not None:
        return _CACHED_NC
    import concourse.tile as tile
    from concourse import bacc, mybir

    F32 = mybir.dt.float32
    F32R = mybir.dt.float32r
    nc = bacc.Bacc("TRN2", target_bir_lowering=False, debug=False)
    xt = nc.dram_tensor("xt", [D, S], F32R, kind="ExternalInput").ap()
    wq = nc.dram_tensor("wq", [128, DC, D], F32R, kind="ExternalInput").ap()
    wk = nc.dram_tensor("wk", [128, DC, D], F32R, kind="ExternalInput").ap()
    wv = nc.dram_tensor("wv", [128, DC, D], F32R, kind="ExternalInput").ap()
    bq = nc.dram_tensor("bq", [128, DC], F32, kind="ExternalInput").ap()
    bv = nc.dram_tensor("bv", [D], F32, kind="ExternalInput").ap()
    out = nc.dram_tensor("out", [HALF, D], F32, kind="ExternalOutput").ap()

    with tile.TileContext(nc) as tc:
        build_kernel(tc, xt, wq, wk, wv, bq, bv, out)
    nc.compile()
    _CACHED_NC = nc
    return nc


def _shard_inputs(x, Wq, bq, Wk, bk, Wv, bv):
    """Host-side prep: per-core permuted x^T + relaid-out weights/biases."""
    wq_r = np.ascontiguousarray(Wq.reshape(DC, 128, D).transpose(1, 0, 2))
    wk_r = np.ascontiguousarray(Wk.T.reshape(DC, 128, D).transpose(1, 0, 2))
    wv_r = np.ascontiguousarray(Wv.reshape(DC, 128, D).transpose(1, 0, 2))
    bq_r = np.ascontiguousarray(bq.reshape(DC, 128).T)
    bv_c = np.ascontiguousarray(bv)

    in_maps = []
    for c in range(NC):
        b, h = divmod(c, 2)
        xb = x[b]
        if h:
            xb = np.concatenate([xb[HALF:], xb[:HALF]], axis=0)
        xt = np.ascontiguousarray(xb.T)  # [D, S], own queries first
        in_maps.append(
            {
                "xt": xt,
                "wq": wq_r,
                "wk": wk_r,
                "wv": wv_r,
                "bq": bq_r,
                "bv": bv_c,
            }
        )
    return in_maps


def kernel(x, Wq, bq, Wk, bk, Wv, bv):
    global LAST_RESULT
    _ensure_axon_ntff_hook()
    from concourse import bass_utils

    x = np.asarray(x, dtype=np.float32)
    args = [np.asarray(a, dtype=np.float32) for a in (Wq, bq, Wk, bk, Wv, bv)]
    nc = build_nc()
    in_maps = _shard_inputs(x, *args)
    res = bass_utils.run_bass_kernel_spmd(nc, in_maps, core_ids=list(range(NC)))
    LAST_RESULT = res
    out = np.empty((B, S, D), dtype=np.float32)
    for c in range(NC):
        b, h = divmod(c, 2)
        out[b, h * HALF : (h + 1) * HALF, :] = res.results[c]["out"]
    return out


if __name__ == "__main__":
    rng = np.random.default_rng(0)
    init = 1.0 / 32.0
    x = rng.standard_normal((B, S, D), dtype=np.float32)
    mk = lambda *s: rng.uniform(-init, init, s).astype(np.float32)
    o = kernel(x, mk(D, D), mk(D), mk(D, D), mk(D), mk(D, D), mk(D))
    print("out", o.shape, o.dtype, float(np.abs(o).max()))


# revision 14
# speedup vs baseline: 1.5961x; 1.1082x over previous
"""Single-head self-attention (B=4, S=2048, D=1024) on 8 trn2 NeuronCores.

Sharding: core c -> (batch b = c//2, query half h = c%2). Each core gets a
permuted x^T for its batch (own seq-half first; softmax is invariant to key
permutation), computes Q for its 1024 queries and K/V for all 2048 keys,
then attention. Output rows are the core's own queries in original order, so
the host gather is a pure concatenation.

Device layout (per core):
  xt (input)  : [1024_d, 2048_t] fp32   (x_perm.T, host-prepared)
  Q^T         : [128_dp, 8_dc, 512_s]   per s-block, SBUF
  K^T         : [128_jp, 8_jc, 2048_t]  spilled to HBM scratch, streamed back
  V           : [128_tp, 16_tc, 1024_j] SBUF-resident
  scores^T    : [128_t, 512_s] PSUM -> exp -> SBUF (keys on partitions, so
                attn@V needs no transposes; softmax sum over keys is an
                extra N=1 matmul vs a ones vector sharing the stationary
                operand; max-subtraction skipped: scores ~ N(0, 0.33))
All matmuls fp32r (fp22 mantissa, full PE rate at N>=256, fp32 accumulate).
"""

import os
import sys
import types

import numpy as np

B, S, D = 4, 2048, 1024
HALF = S // 2  # 1024 queries per core
SCALE = 1.0 / 32.0  # 1/sqrt(D)
NC = 8
DC = D // 128  # 8 d-chunks
TT = S // 128  # 16 key tiles
SBLK = 512  # queries per s-block
NSB = HALF // SBLK  # 2 s-blocks

_CACHED_NC = None
LAST_RESULT = None  # BassKernelResults of the most recent run (for test.py)


def _ensure_axon_ntff_hook():
    """bass_utils' trace path needs antenv.axon_hooks; this image's antenv
    lacks it. Install a shim backed by trn_agent_boot's ctypes hook so
    BASS_TRACE=1 profiling works. No-op if already present/unavailable."""
    try:
        import antenv.axon_hooks  # noqa: F401

        return
    except ImportError:
        pass
    try:
        from trn_agent_boot.trn_boot import _ntff_profile_via_ctypes

        hook = _ntff_profile_via_ctypes("/opt/axon/libaxon_pjrt.so")
    except Exception:
        hook = None
    mod = types.ModuleType("antenv.axon_hooks")
    mod.get_axon_ntff_profile_hook = lambda: hook
    mod.set_axon_ntff_profile_hook = lambda h: None
    sys.modules["antenv.axon_hooks"] = mod


def build_kernel(tc, xt, xn, wq, wk, wv, bq, bv, out):
    """Per-core attention with neither K^T nor V materialized:
      Q^T = Wq-proj of own queries (+bq)            [128, DC, 1024_s]
      G   = Wk @ Q^T   (K-side projection applied to the small Q side)
      scores^T[t, s] = sum_d xT[d, t] G[d, s]       (K bias cancels)
      H^T[d, s] = sum_t x[t, d] expP[t, s]          (attn contracts x first)
      out[s, j] = (sum_d H^T[d, s] Wv[d, j]) / l[s] + bv[j]
    This removes every duplicated projection: 15.05 GFLOP/core, the exact
    1/8 share of the network's total work.
    """
    import concourse.bass as bass
    from concourse import mybir

    nc = tc.nc
    F32 = mybir.dt.float32
    F32R = mybir.dt.float32r
    Identity = mybir.ActivationFunctionType.Identity
    Copy = mybir.ActivationFunctionType.Copy
    Exp = mybir.ActivationFunctionType.Exp

    xt_r = xt.rearrange("(c p) t -> p c t", p=128)  # [128, 8, 2048]
    xn_r = xn.rearrange("(tc p) d -> p tc d", p=128)  # [128, 16, 1024]
    out_r = out.rearrange("(su p) j -> su p j", p=128)  # [8, 128, 1024]

    with tc.tile_pool(name="persist", bufs=1) as persist:
        xT = persist.tile([128, DC, S], F32R)
        G = persist.tile([128, DC, HALF], F32R)
        bv_bc = persist.tile([128, D], F32)
        bv_bcast_ap = bass.AP(
            tensor=bv.tensor, offset=bv.offset, ap=[[0, 128]] + list(bv.ap)
        )
        nc.scalar.dma_start(bv_bc, bv_bcast_ap)
        bq_sb = persist.tile([128, DC], F32)
        nc.scalar.dma_start(bq_sb, bq)
        ones_f = persist.tile([128, 2], F32)
        nc.vector.memset(ones_f, 1.0)
        ones_t = persist.tile([128, 2], F32R)
        nc.vector.tensor_copy(ones_t, ones_f)

        # x^T loaded t-block-major so early query groups unblock first
        for tb in range(S // 512):
            for c in range(DC):
                nc.sync.dma_start(
                    xT[:, c, tb * 512 : (tb + 1) * 512],
                    xt_r[:, c, tb * 512 : (tb + 1) * 512],
                )

        # ---- Phase A: Q^T then G = Wk @ Q^T ------------------------------
        with (
            tc.tile_pool(name="pa1", bufs=1) as pa1,
            tc.tile_pool(name="pa_w", bufs=2) as paw,
            tc.tile_pool(name="psa", bufs=2, space="PSUM") as psa,
        ):
            qT = pa1.tile([128, DC, HALF], F32R)
            for qc in range(DC):
                wq_t = paw.tile([128, DC, 128], F32R, tag="w_t")
                nc.scalar.dma_start(wq_t, wq[:, :, qc * 128 : (qc + 1) * 128])
                for sblk in range(NSB):
                    qpsum = psa.tile([128, SBLK], F32, tag="qpsum")
                    for c in range(DC):
                        nc.tensor.matmul(
                            qpsum,
                            wq_t[:, c, :],
                            xT[:, c, sblk * SBLK : (sblk + 1) * SBLK],
                            start=(c == 0),
                            stop=(c == DC - 1),
                        )
                    nc.scalar.activation(
                        qT[:, qc, sblk * SBLK : (sblk + 1) * SBLK],
                        qpsum,
                        Identity,
                        bias=bq_sb[:, qc : qc + 1],
                    )

            # G[d, s] = sum_j Wk[d, j] qT[j, s]  (wk passed j-major = Wk.T)
            for gc in range(DC):
                wk_t = paw.tile([128, DC, 128], F32R, tag="w_t")
                nc.scalar.dma_start(wk_t, wk[:, :, gc * 128 : (gc + 1) * 128])
                for sblk in range(NSB):
                    gpsum = psa.tile([128, SBLK], F32, tag="gpsum")
                    for jc in range(DC):
                        nc.tensor.matmul(
                            gpsum,
                            wk_t[:, jc, :],
                            qT[:, jc, sblk * SBLK : (sblk + 1) * SBLK],
                            start=(jc == 0),
                            stop=(jc == DC - 1),
                        )
                    nc.scalar.activation(
                        G[:, gc, sblk * SBLK : (sblk + 1) * SBLK], gpsum, Copy
                    )

        # ---- Phase B: scores^T -> exp -> H^T -> out, per 512-query block -
        with (
            tc.tile_pool(name="pb_wv", bufs=1) as pbwv,
            tc.tile_pool(name="pb_p", bufs=1) as pbp,
            tc.tile_pool(name="pb_x", bufs=2) as pbx,
            tc.tile_pool(name="pb_h", bufs=1) as pbh,
            tc.tile_pool(name="pb_o", bufs=2) as pbo,
            tc.tile_pool(name="pb_m", bufs=2) as pbm,
            tc.tile_pool(name="psb_s", bufs=2, space="PSUM") as psbs,
            tc.tile_pool(name="psb_h", bufs=2, space="PSUM") as psbh,
            tc.tile_pool(name="psb_o", bufs=2, space="PSUM") as psbo,
            tc.tile_pool(name="psb_l", bufs=2, space="PSUM") as psbl,
        ):
            wv_sb = pbwv.tile([128, DC, D], F32R)
            nc.scalar.dma_start(wv_sb, wv)
            for sb in range(NSB):
                # scores^T + exp; E accumulates the softmax sums on DVE
                expP = pbp.tile([128, TT, SBLK], F32R, tag="expP")
                E_t = pbp.tile([128, SBLK], F32R, tag="E_t", bufs=1)
                for tt in range(TT):
                    spsum = psbs.tile([128, SBLK], F32, tag="spsum")
                    for c in range(DC):
                        nc.tensor.matmul(
                            spsum,
                            xT[:, c, tt * 128 : (tt + 1) * 128],
                            G[:, c, sb * SBLK : (sb + 1) * SBLK],
                            start=(c == 0),
                            stop=(c == DC - 1),
                        )
                    nc.scalar.activation(expP[:, tt, :], spsum, Exp, scale=SCALE)
                    if tt == 1:
                        nc.vector.tensor_add(E_t, expP[:, 0, :], expP[:, 1, :])
                    elif tt > 1:
                        nc.vector.tensor_add(E_t, E_t, expP[:, tt, :])

                # H^T[d, s] = sum_t x[t, d] expP[t, s]
                H = pbh.tile([128, DC, SBLK], F32R, tag="H")
                for dc in range(DC):
                    xn_t = pbx.tile([128, TT, 128], F32R, tag="xn_t")
                    nc.sync.dma_start(xn_t, xn_r[:, :, dc * 128 : (dc + 1) * 128])
                    hpsum = psbh.tile([128, SBLK], F32, tag="hpsum")
                    for tt in range(TT):
                        nc.tensor.matmul(
                            hpsum,
                            xn_t[:, tt, :],
                            expP[:, tt, :],
                            start=(tt == 0),
                            stop=(tt == TT - 1),
                        )
                    nc.scalar.activation(H[:, dc, :], hpsum, Copy)

                # out[s, j] = (sum_d H^T[d, s] Wv[d, j]) / l[s] + bv[j]
                for su in range(SBLK // 128):
                    s0 = su * 128
                    lpsum = psbl.tile([128, 2], F32, tag="lpsum")
                    nc.tensor.matmul(
                        lpsum, E_t[:, s0 : s0 + 128], ones_t, start=True, stop=True
                    )
                    recip = pbm.tile([128, 1], F32, tag="recip")
                    nc.vector.reciprocal(recip, lpsum[:, 0:1])
                    for jb in range(2):
                        opsum = psbo.tile([128, 512], F32, tag="opsum")
                        for dc in range(DC):
                            nc.tensor.matmul(
                                opsum,
                                H[:, dc, s0 : s0 + 128],
                                wv_sb[:, dc, jb * 512 : (jb + 1) * 512],
                                start=(dc == 0),
                                stop=(dc == DC - 1),
                            )
                        o_sb = pbo.tile([128, 512], F32, tag="o_sb")
                        nc.vector.tensor_scalar_mul(o_sb, in0=opsum, scalar1=recip)
                        nc.vector.tensor_add(
                            o_sb, o_sb, bv_bc[:, jb * 512 : (jb + 1) * 512]
                        )
                        nc.sync.dma_start(
                            out_r[sb * (SBLK // 128) + su][
                                :, jb * 512 : (jb + 1) * 512
                            ],
                            o_sb,
                        )


def build_nc():
    global _CACHED_NC
    if _CACHED_NC is not None:
        return _CACHED_NC
    import concourse.tile as tile
    from concourse import bacc, mybir

    F32 = mybir.dt.float32
    F32R = mybir.dt.float32r
    nc = bacc.Bacc("TRN2", target_bir_lowering=False, debug=False)
    xt = nc.dram_tensor("xt", [D, S], F32R, kind="ExternalInput").ap()
    xn = nc.dram_tensor("xn", [S, D], F32R, kind="ExternalInput").ap()
    wq = nc.dram_tensor("wq", [128, DC, D], F32R, kind="ExternalInput").ap()
    wk = nc.dram_tensor("wk", [128, DC, D], F32R, kind="ExternalInput").ap()
    wv = nc.dram_tensor("wv", [128, DC, D], F32R, kind="ExternalInput").ap()
    bq = nc.dram_tensor("bq", [128, DC], F32, kind="ExternalInput").ap()
    bv = nc.dram_tensor("bv", [D], F32, kind="ExternalInput").ap()
    out = nc.dram_tensor("out", [HALF, D], F32, kind="ExternalOutput").ap()

    with tile.TileContext(nc) as tc:
        build_kernel(tc, xt, xn, wq, wk, wv, bq, bv, out)
    nc.compile()
    _CACHED_NC = nc
    return nc


def _shard_inputs(x, Wq, bq, Wk, bk, Wv, bv):
    """Host-side prep: per-core permuted x^T + relaid-out weights/biases."""
    wq_r = np.ascontiguousarray(Wq.reshape(DC, 128, D).transpose(1, 0, 2))
    wk_r = np.ascontiguousarray(Wk.T.reshape(DC, 128, D).transpose(1, 0, 2))
    wv_r = np.ascontiguousarray(Wv.reshape(DC, 128, D).transpose(1, 0, 2))
    bq_r = np.ascontiguousarray(bq.reshape(DC, 128).T)
    bv_c = np.ascontiguousarray(bv)

    in_maps = []
    for c in range(NC):
        b, h = divmod(c, 2)
        xb = x[b]
        if h:
            xb = np.concatenate([xb[HALF:], xb[:HALF]], axis=0)
        xt = np.ascontiguousarray(xb.T)  # [D, S], own queries first
        xn = np.ascontiguousarray(xb)  # [S, D], same permutation
        in_maps.append(
            {
                "xt": xt,
                "xn": xn,
                "wq": wq_r,
                "wk": wk_r,
                "wv": wv_r,
                "bq": bq_r,
                "bv": bv_c,
            }
        )
    return in_maps


def kernel(x, Wq, bq, Wk, bk, Wv, bv):
    global LAST_RESULT
    _ensure_axon_ntff_hook()
    from concourse import bass_utils

    x = np.asarray(x, dtype=np.float32)
    args = [np.asarray(a, dtype=np.float32) for a in (Wq, bq, Wk, bk, Wv, bv)]
    nc = build_nc()
    in_maps = _shard_inputs(x, *args)
    res = bass_utils.run_bass_kernel_spmd(nc, in_maps, core_ids=list(range(NC)))
    LAST_RESULT = res
    out = np.empty((B, S, D), dtype=np.float32)
    for c in range(NC):
        b, h = divmod(c, 2)
        out[b, h * HALF : (h + 1) * HALF, :] = res.results[c]["out"]
    return out


if __name__ == "__main__":
    rng = np.random.default_rng(0)
    init = 1.0 / 32.0
    x = rng.standard_normal((B, S, D), dtype=np.float32)
    mk = lambda *s: rng.uniform(-init, init, s).astype(np.float32)
    o = kernel(x, mk(D, D), mk(D), mk(D, D), mk(D), mk(D, D), mk(D))
    print("out", o.shape, o.dtype, float(np.abs(o).max()))


# revision 15
# speedup vs baseline: 1.6054x; 1.0058x over previous
"""Single-head self-attention (B=4, S=2048, D=1024) on 8 trn2 NeuronCores.

Sharding: core c -> (batch b = c//2, query half h = c%2). Each core gets a
permuted x^T for its batch (own seq-half first; softmax is invariant to key
permutation), computes Q for its 1024 queries and K/V for all 2048 keys,
then attention. Output rows are the core's own queries in original order, so
the host gather is a pure concatenation.

Device layout (per core):
  xt (input)  : [1024_d, 2048_t] fp32   (x_perm.T, host-prepared)
  Q^T         : [128_dp, 8_dc, 512_s]   per s-block, SBUF
  K^T         : [128_jp, 8_jc, 2048_t]  spilled to HBM scratch, streamed back
  V           : [128_tp, 16_tc, 1024_j] SBUF-resident
  scores^T    : [128_t, 512_s] PSUM -> exp -> SBUF (keys on partitions, so
                attn@V needs no transposes; softmax sum over keys is an
                extra N=1 matmul vs a ones vector sharing the stationary
                operand; max-subtraction skipped: scores ~ N(0, 0.33))
All matmuls fp32r (fp22 mantissa, full PE rate at N>=256, fp32 accumulate).
"""

import os
import sys
import types

import numpy as np

B, S, D = 4, 2048, 1024
HALF = S // 2  # 1024 queries per core
SCALE = 1.0 / 32.0  # 1/sqrt(D)
NC = 8
DC = D // 128  # 8 d-chunks
TT = S // 128  # 16 key tiles
SBLK = 512  # queries per s-block
NSB = HALF // SBLK  # 2 s-blocks

_CACHED_NC = None
LAST_RESULT = None  # BassKernelResults of the most recent run (for test.py)


def _ensure_axon_ntff_hook():
    """bass_utils' trace path needs antenv.axon_hooks; this image's antenv
    lacks it. Install a shim backed by trn_agent_boot's ctypes hook so
    BASS_TRACE=1 profiling works. No-op if already present/unavailable."""
    try:
        import antenv.axon_hooks  # noqa: F401

        return
    except ImportError:
        pass
    try:
        from trn_agent_boot.trn_boot import _ntff_profile_via_ctypes

        hook = _ntff_profile_via_ctypes("/opt/axon/libaxon_pjrt.so")
    except Exception:
        hook = None
    mod = types.ModuleType("antenv.axon_hooks")
    mod.get_axon_ntff_profile_hook = lambda: hook
    mod.set_axon_ntff_profile_hook = lambda h: None
    sys.modules["antenv.axon_hooks"] = mod


def build_kernel(tc, xt, xn, wq, wk, wv, bq, bv, out):
    """Per-core attention with neither K^T nor V materialized:
      Q^T = Wq-proj of own queries (+bq)            [128, DC, 1024_s]
      G   = Wk @ Q^T   (K-side projection applied to the small Q side)
      scores^T[t, s] = sum_d xT[d, t] G[d, s]       (K bias cancels)
      H^T[d, s] = sum_t x[t, d] expP[t, s]          (attn contracts x first)
      out[s, j] = (sum_d H^T[d, s] Wv[d, j]) / l[s] + bv[j]
    This removes every duplicated projection: 15.05 GFLOP/core, the exact
    1/8 share of the network's total work.
    """
    import concourse.bass as bass
    from concourse import mybir

    nc = tc.nc
    F32 = mybir.dt.float32
    F32R = mybir.dt.float32r
    Identity = mybir.ActivationFunctionType.Identity
    Copy = mybir.ActivationFunctionType.Copy
    Exp = mybir.ActivationFunctionType.Exp

    xt_r = xt.rearrange("(c p) t -> p c t", p=128)  # [128, 8, 2048]
    xn_r = xn.rearrange("(tc p) d -> p tc d", p=128)  # [128, 16, 1024]
    out_r = out.rearrange("(su p) j -> su p j", p=128)  # [8, 128, 1024]

    with tc.tile_pool(name="persist", bufs=1) as persist:
        xT = persist.tile([128, DC, S], F32R)
        G = persist.tile([128, DC, HALF], F32R)
        bv_bc = persist.tile([128, D], F32)
        bv_bcast_ap = bass.AP(
            tensor=bv.tensor, offset=bv.offset, ap=[[0, 128]] + list(bv.ap)
        )
        nc.gpsimd.dma_start(bv_bc, bv_bcast_ap)
        bq_sb = persist.tile([128, DC], F32)
        nc.gpsimd.dma_start(bq_sb, bq)
        ones_f = persist.tile([128, 2], F32)
        nc.vector.memset(ones_f, 1.0)
        ones_t = persist.tile([128, 2], F32R)
        nc.vector.tensor_copy(ones_t, ones_f)

        # x^T loaded t-block-major so early query groups unblock first
        for tb in range(S // 512):
            for c in range(DC):
                nc.sync.dma_start(
                    xT[:, c, tb * 512 : (tb + 1) * 512],
                    xt_r[:, c, tb * 512 : (tb + 1) * 512],
                )

        # ---- Phase A: Q^T then G = Wk @ Q^T ------------------------------
        with (
            tc.tile_pool(name="pa1", bufs=1) as pa1,
            tc.tile_pool(name="pa_w", bufs=2) as paw,
            tc.tile_pool(name="psa", bufs=2, space="PSUM") as psa,
        ):
            qT = pa1.tile([128, DC, HALF], F32R)
            for qc in range(DC):
                wq_t = paw.tile([128, DC, 128], F32R, tag="w_t")
                nc.gpsimd.dma_start(wq_t, wq[:, :, qc * 128 : (qc + 1) * 128])
                for sblk in range(NSB):
                    qpsum = psa.tile([128, SBLK], F32, tag="qpsum")
                    for c in range(DC):
                        nc.tensor.matmul(
                            qpsum,
                            wq_t[:, c, :],
                            xT[:, c, sblk * SBLK : (sblk + 1) * SBLK],
                            start=(c == 0),
                            stop=(c == DC - 1),
                        )
                    nc.scalar.activation(
                        qT[:, qc, sblk * SBLK : (sblk + 1) * SBLK],
                        qpsum,
                        Identity,
                        bias=bq_sb[:, qc : qc + 1],
                    )

            # G[d, s] = sum_j Wk[d, j] qT[j, s]  (wk passed j-major = Wk.T)
            for gc in range(DC):
                wk_t = paw.tile([128, DC, 128], F32R, tag="w_t")
                nc.gpsimd.dma_start(wk_t, wk[:, :, gc * 128 : (gc + 1) * 128])
                for sblk in range(NSB):
                    gpsum = psa.tile([128, SBLK], F32, tag="gpsum")
                    for jc in range(DC):
                        nc.tensor.matmul(
                            gpsum,
                            wk_t[:, jc, :],
                            qT[:, jc, sblk * SBLK : (sblk + 1) * SBLK],
                            start=(jc == 0),
                            stop=(jc == DC - 1),
                        )
                    nc.scalar.activation(
                        G[:, gc, sblk * SBLK : (sblk + 1) * SBLK], gpsum, Copy
                    )

        # ---- Phase B: scores^T -> exp -> H^T -> out, per 512-query block -
        with (
            tc.tile_pool(name="pb_wv", bufs=1) as pbwv,
            tc.tile_pool(name="pb_p", bufs=1) as pbp,
            tc.tile_pool(name="pb_x", bufs=2) as pbx,
            tc.tile_pool(name="pb_h", bufs=1) as pbh,
            tc.tile_pool(name="pb_o", bufs=2) as pbo,
            tc.tile_pool(name="pb_m", bufs=2) as pbm,
            tc.tile_pool(name="psb_s", bufs=2, space="PSUM") as psbs,
            tc.tile_pool(name="psb_h", bufs=2, space="PSUM") as psbh,
            tc.tile_pool(name="psb_o", bufs=2, space="PSUM") as psbo,
            tc.tile_pool(name="psb_l", bufs=2, space="PSUM") as psbl,
        ):
            wv_sb = pbwv.tile([128, DC, D], F32R)
            nc.gpsimd.dma_start(wv_sb, wv)
            for sb in range(NSB):
                # scores^T + exp; E accumulates the softmax sums on DVE
                expP = pbp.tile([128, TT, SBLK], F32R, tag="expP")
                E_t = pbp.tile([128, SBLK], F32R, tag="E_t", bufs=1)
                for tt in range(TT):
                    spsum = psbs.tile([128, SBLK], F32, tag="spsum")
                    for c in range(DC):
                        nc.tensor.matmul(
                            spsum,
                            xT[:, c, tt * 128 : (tt + 1) * 128],
                            G[:, c, sb * SBLK : (sb + 1) * SBLK],
                            start=(c == 0),
                            stop=(c == DC - 1),
                        )
                    nc.scalar.activation(expP[:, tt, :], spsum, Exp, scale=SCALE)
                    if tt == 1:
                        nc.vector.tensor_add(E_t, expP[:, 0, :], expP[:, 1, :])
                    elif tt > 1:
                        nc.vector.tensor_add(E_t, E_t, expP[:, tt, :])

                # H^T[d, s] = sum_t x[t, d] expP[t, s]
                H = pbh.tile([128, DC, SBLK], F32R, tag="H")
                for dc in range(DC):
                    xn_t = pbx.tile([128, TT, 128], F32R, tag="xn_t")
                    nc.sync.dma_start(xn_t, xn_r[:, :, dc * 128 : (dc + 1) * 128])
                    hpsum = psbh.tile([128, SBLK], F32, tag="hpsum")
                    for tt in range(TT):
                        nc.tensor.matmul(
                            hpsum,
                            xn_t[:, tt, :],
                            expP[:, tt, :],
                            start=(tt == 0),
                            stop=(tt == TT - 1),
                        )
                    nc.scalar.activation(H[:, dc, :], hpsum, Copy)

                # out[s, j] = (sum_d H^T[d, s] Wv[d, j]) / l[s] + bv[j]
                for su in range(SBLK // 128):
                    s0 = su * 128
                    lpsum = psbl.tile([128, 2], F32, tag="lpsum")
                    nc.tensor.matmul(
                        lpsum, E_t[:, s0 : s0 + 128], ones_t, start=True, stop=True
                    )
                    recip = pbm.tile([128, 1], F32, tag="recip")
                    nc.vector.reciprocal(recip, lpsum[:, 0:1])
                    for jb in range(2):
                        opsum = psbo.tile([128, 512], F32, tag="opsum")
                        for dc in range(DC):
                            nc.tensor.matmul(
                                opsum,
                                H[:, dc, s0 : s0 + 128],
                                wv_sb[:, dc, jb * 512 : (jb + 1) * 512],
                                start=(dc == 0),
                                stop=(dc == DC - 1),
                            )
                        o_sb = pbo.tile([128, 512], F32, tag="o_sb")
                        nc.vector.tensor_scalar_mul(o_sb, in0=opsum, scalar1=recip)
                        nc.vector.tensor_add(
                            o_sb, o_sb, bv_bc[:, jb * 512 : (jb + 1) * 512]
                        )
                        nc.sync.dma_start(
                            out_r[sb * (SBLK // 128) + su][
                                :, jb * 512 : (jb + 1) * 512
                            ],
                            o_sb,
                        )


def build_nc():
    global _CACHED_NC
    if _CACHED_NC is not None:
        return _CACHED_NC
    import concourse.tile as tile
    from concourse import bacc, mybir

    F32 = mybir.dt.float32
    F32R = mybir.dt.float32r
    nc = bacc.Bacc("TRN2", target_bir_lowering=False, debug=False)
    xt = nc.dram_tensor("xt", [D, S], F32R, kind="ExternalInput").ap()
    xn = nc.dram_tensor("xn", [S, D], F32R, kind="ExternalInput").ap()
    wq = nc.dram_tensor("wq", [128, DC, D], F32R, kind="ExternalInput").ap()
    wk = nc.dram_tensor("wk", [128, DC, D], F32R, kind="ExternalInput").ap()
    wv = nc.dram_tensor("wv", [128, DC, D], F32R, kind="ExternalInput").ap()
    bq = nc.dram_tensor("bq", [128, DC], F32, kind="ExternalInput").ap()
    bv = nc.dram_tensor("bv", [D], F32, kind="ExternalInput").ap()
    out = nc.dram_tensor("out", [HALF, D], F32, kind="ExternalOutput").ap()

    with tile.TileContext(nc) as tc:
        build_kernel(tc, xt, xn, wq, wk, wv, bq, bv, out)
    nc.compile()
    _CACHED_NC = nc
    return nc


def _shard_inputs(x, Wq, bq, Wk, bk, Wv, bv):
    """Host-side prep: per-core permuted x^T + relaid-out weights/biases."""
    wq_r = np.ascontiguousarray(Wq.reshape(DC, 128, D).transpose(1, 0, 2))
    wk_r = np.ascontiguousarray(Wk.T.reshape(DC, 128, D).transpose(1, 0, 2))
    wv_r = np.ascontiguousarray(Wv.reshape(DC, 128, D).transpose(1, 0, 2))
    bq_r = np.ascontiguousarray(bq.reshape(DC, 128).T)
    bv_c = np.ascontiguousarray(bv)

    in_maps = []
    for c in range(NC):
        b, h = divmod(c, 2)
        xb = x[b]
        if h:
            xb = np.concatenate([xb[HALF:], xb[:HALF]], axis=0)
        xt = np.ascontiguousarray(xb.T)  # [D, S], own queries first
        xn = np.ascontiguousarray(xb)  # [S, D], same permutation
        in_maps.append(
            {
                "xt": xt,
                "xn": xn,
                "wq": wq_r,
                "wk": wk_r,
                "wv": wv_r,
                "bq": bq_r,
                "bv": bv_c,
            }
        )
    return in_maps


def kernel(x, Wq, bq, Wk, bk, Wv, bv):
    global LAST_RESULT
    _ensure_axon_ntff_hook()
    from concourse import bass_utils

    x = np.asarray(x, dtype=np.float32)
    args = [np.asarray(a, dtype=np.float32) for a in (Wq, bq, Wk, bk, Wv, bv)]
    nc = build_nc()
    in_maps = _shard_inputs(x, *args)
    res = bass_utils.run_bass_kernel_spmd(nc, in_maps, core_ids=list(range(NC)))
    LAST_RESULT = res
    out = np.empty((B, S, D), dtype=np.float32)
    for c in range(NC):
        b, h = divmod(c, 2)
        out[b, h * HALF : (h + 1) * HALF, :] = res.results[c]["out"]
    return out


if __name__ == "__main__":
    rng = np.random.default_rng(0)
    init = 1.0 / 32.0
    x = rng.standard_normal((B, S, D), dtype=np.float32)
    mk = lambda *s: rng.uniform(-init, init, s).astype(np.float32)
    o = kernel(x, mk(D, D), mk(D), mk(D, D), mk(D), mk(D, D), mk(D))
    print("out", o.shape, o.dtype, float(np.abs(o).max()))


# revision 16
# speedup vs baseline: 1.8212x; 1.1344x over previous
"""Single-head self-attention (B=4, S=2048, D=1024) on 8 trn2 NeuronCores.

Sharding: core c -> (batch b = c//2, query half h = c%2). Each core gets a
permuted x^T for its batch (own seq-half first; softmax is invariant to key
permutation), computes Q for its 1024 queries and K/V for all 2048 keys,
then attention. Output rows are the core's own queries in original order, so
the host gather is a pure concatenation.

Device layout (per core):
  xt (input)  : [1024_d, 2048_t] fp32   (x_perm.T, host-prepared)
  Q^T         : [128_dp, 8_dc, 512_s]   per s-block, SBUF
  K^T         : [128_jp, 8_jc, 2048_t]  spilled to HBM scratch, streamed back
  V           : [128_tp, 16_tc, 1024_j] SBUF-resident
  scores^T    : [128_t, 512_s] PSUM -> exp -> SBUF (keys on partitions, so
                attn@V needs no transposes; softmax sum over keys is an
                extra N=1 matmul vs a ones vector sharing the stationary
                operand; max-subtraction skipped: scores ~ N(0, 0.33))
All matmuls fp32r (fp22 mantissa, full PE rate at N>=256, fp32 accumulate).
"""

import os
import sys
import types

import numpy as np

B, S, D = 4, 2048, 1024
HALF = S // 2  # 1024 queries per core
SCALE = 1.0 / 32.0  # 1/sqrt(D)
NC = 8
DC = D // 128  # 8 d-chunks
TT = S // 128  # 16 key tiles
SBLK = 512  # queries per s-block
NSB = HALF // SBLK  # 2 s-blocks

_CACHED_NC = None
LAST_RESULT = None  # BassKernelResults of the most recent run (for test.py)


def _ensure_axon_ntff_hook():
    """bass_utils' trace path needs antenv.axon_hooks; this image's antenv
    lacks it. Install a shim backed by trn_agent_boot's ctypes hook so
    BASS_TRACE=1 profiling works. No-op if already present/unavailable."""
    try:
        import antenv.axon_hooks  # noqa: F401

        return
    except ImportError:
        pass
    try:
        from trn_agent_boot.trn_boot import _ntff_profile_via_ctypes

        hook = _ntff_profile_via_ctypes("/opt/axon/libaxon_pjrt.so")
    except Exception:
        hook = None
    mod = types.ModuleType("antenv.axon_hooks")
    mod.get_axon_ntff_profile_hook = lambda: hook
    mod.set_axon_ntff_profile_hook = lambda h: None
    sys.modules["antenv.axon_hooks"] = mod


def build_kernel(tc, xt, xn, wq, wk, wv, bq, bv, out):
    """Per-core attention with neither K^T nor V materialized:
      Q^T = Wq-proj of own queries (+bq)            [128, DC, 1024_s]
      G   = Wk @ Q^T   (K-side projection applied to the small Q side)
      scores^T[t, s] = sum_d xT[d, t] G[d, s]       (K bias cancels)
      H^T[d, s] = sum_t x[t, d] expP[t, s]          (attn contracts x first)
      out[s, j] = (sum_d H^T[d, s] Wv[d, j]) / l[s] + bv[j]
    This removes every duplicated projection: 15.05 GFLOP/core, the exact
    1/8 share of the network's total work.
    """
    import concourse.bass as bass
    from concourse import mybir

    nc = tc.nc
    F32 = mybir.dt.float32
    F32R = mybir.dt.float32r
    Identity = mybir.ActivationFunctionType.Identity
    Copy = mybir.ActivationFunctionType.Copy
    Exp = mybir.ActivationFunctionType.Exp

    xt_r = xt.rearrange("(c p) t -> p c t", p=128)  # [128, 8, 2048]
    xn_r = xn.rearrange("(tc p) d -> p tc d", p=128)  # [128, 16, 1024]
    out_r = out.rearrange("(su p) j -> su p j", p=128)  # [8, 128, 1024]

    with tc.tile_pool(name="persist", bufs=1) as persist:
        xT = persist.tile([128, DC, S], F32R)
        G = persist.tile([128, DC, HALF], F32R)
        bv_bc = persist.tile([128, D], F32)
        bv_bcast_ap = bass.AP(
            tensor=bv.tensor, offset=bv.offset, ap=[[0, 128]] + list(bv.ap)
        )
        nc.scalar.dma_start(bv_bc, bv_bcast_ap)
        bq_sb = persist.tile([128, DC], F32)
        nc.scalar.dma_start(bq_sb, bq)
        ones_f = persist.tile([128, 2], F32)
        nc.vector.memset(ones_f, 1.0)
        ones_t = persist.tile([128, 2], F32R)
        nc.vector.tensor_copy(ones_t, ones_f)

        # x^T loaded t-block-major so early query groups unblock first
        for tb in range(S // 512):
            for c in range(DC):
                nc.sync.dma_start(
                    xT[:, c, tb * 512 : (tb + 1) * 512],
                    xt_r[:, c, tb * 512 : (tb + 1) * 512],
                )

        # ---- Phase A: Q^T then G = Wk @ Q^T ------------------------------
        with (
            tc.tile_pool(name="pa1", bufs=1) as pa1,
            tc.tile_pool(name="pa_w", bufs=6) as paw,
            tc.tile_pool(name="psa", bufs=2, space="PSUM") as psa,
        ):
            qT = pa1.tile([128, DC, HALF], F32R)
            for qc in range(DC):
                wq_t = paw.tile([128, DC, 128], F32R, tag="w_t")
                nc.scalar.dma_start(wq_t, wq[:, :, qc * 128 : (qc + 1) * 128])
                for sblk in range(NSB):
                    qpsum = psa.tile([128, SBLK], F32, tag="qpsum")
                    for c in range(DC):
                        nc.tensor.matmul(
                            qpsum,
                            wq_t[:, c, :],
                            xT[:, c, sblk * SBLK : (sblk + 1) * SBLK],
                            start=(c == 0),
                            stop=(c == DC - 1),
                        )
                    nc.scalar.activation(
                        qT[:, qc, sblk * SBLK : (sblk + 1) * SBLK],
                        qpsum,
                        Identity,
                        bias=bq_sb[:, qc : qc + 1],
                    )

            # G[d, s] = sum_j Wk[d, j] qT[j, s]  (wk passed j-major = Wk.T)
            for gc in range(DC):
                wk_t = paw.tile([128, DC, 128], F32R, tag="w_t")
                nc.scalar.dma_start(wk_t, wk[:, :, gc * 128 : (gc + 1) * 128])
                for sblk in range(NSB):
                    gpsum = psa.tile([128, SBLK], F32, tag="gpsum")
                    for jc in range(DC):
                        nc.tensor.matmul(
                            gpsum,
                            wk_t[:, jc, :],
                            qT[:, jc, sblk * SBLK : (sblk + 1) * SBLK],
                            start=(jc == 0),
                            stop=(jc == DC - 1),
                        )
                    nc.scalar.activation(
                        G[:, gc, sblk * SBLK : (sblk + 1) * SBLK], gpsum, Copy
                    )

        # ---- Phase B: scores^T -> exp -> H^T -> out, per 512-query block -
        with (
            tc.tile_pool(name="pb_wv", bufs=1) as pbwv,
            tc.tile_pool(name="pb_p", bufs=1) as pbp,
            tc.tile_pool(name="pb_x", bufs=2) as pbx,
            tc.tile_pool(name="pb_h", bufs=1) as pbh,
            tc.tile_pool(name="pb_o", bufs=2) as pbo,
            tc.tile_pool(name="pb_m", bufs=2) as pbm,
            tc.tile_pool(name="psb_s", bufs=2, space="PSUM") as psbs,
            tc.tile_pool(name="psb_h", bufs=2, space="PSUM") as psbh,
            tc.tile_pool(name="psb_o", bufs=2, space="PSUM") as psbo,
            tc.tile_pool(name="psb_l", bufs=2, space="PSUM") as psbl,
        ):
            wv_sb = pbwv.tile([128, DC, D], F32R)
            nc.gpsimd.dma_start(wv_sb, wv)
            for sb in range(NSB):
                # scores^T + exp; E accumulates the softmax sums on DVE
                expP = pbp.tile([128, TT, SBLK], F32R, tag="expP")
                E_t = pbp.tile([128, SBLK], F32R, tag="E_t", bufs=1)
                for tt in range(TT):
                    spsum = psbs.tile([128, SBLK], F32, tag="spsum")
                    for c in range(DC):
                        nc.tensor.matmul(
                            spsum,
                            xT[:, c, tt * 128 : (tt + 1) * 128],
                            G[:, c, sb * SBLK : (sb + 1) * SBLK],
                            start=(c == 0),
                            stop=(c == DC - 1),
                        )
                    nc.scalar.activation(expP[:, tt, :], spsum, Exp, scale=SCALE)
                    if tt == 1:
                        nc.vector.tensor_add(E_t, expP[:, 0, :], expP[:, 1, :])
                    elif tt > 1:
                        nc.vector.tensor_add(E_t, E_t, expP[:, tt, :])

                # H^T[d, s] = sum_t x[t, d] expP[t, s]
                H = pbh.tile([128, DC, SBLK], F32R, tag="H")
                for dc in range(DC):
                    xn_t = pbx.tile([128, TT, 128], F32R, tag="xn_t")
                    nc.sync.dma_start(xn_t, xn_r[:, :, dc * 128 : (dc + 1) * 128])
                    hpsum = psbh.tile([128, SBLK], F32, tag="hpsum")
                    for tt in range(TT):
                        nc.tensor.matmul(
                            hpsum,
                            xn_t[:, tt, :],
                            expP[:, tt, :],
                            start=(tt == 0),
                            stop=(tt == TT - 1),
                        )
                    nc.scalar.activation(H[:, dc, :], hpsum, Copy)

                # out[s, j] = (sum_d H^T[d, s] Wv[d, j]) / l[s] + bv[j]
                for su in range(SBLK // 128):
                    s0 = su * 128
                    lpsum = psbl.tile([128, 2], F32, tag="lpsum")
                    nc.tensor.matmul(
                        lpsum, E_t[:, s0 : s0 + 128], ones_t, start=True, stop=True
                    )
                    recip = pbm.tile([128, 1], F32, tag="recip")
                    nc.vector.reciprocal(recip, lpsum[:, 0:1])
                    for jb in range(2):
                        opsum = psbo.tile([128, 512], F32, tag="opsum")
                        for dc in range(DC):
                            nc.tensor.matmul(
                                opsum,
                                H[:, dc, s0 : s0 + 128],
                                wv_sb[:, dc, jb * 512 : (jb + 1) * 512],
                                start=(dc == 0),
                                stop=(dc == DC - 1),
                            )
                        o_sb = pbo.tile([128, 512], F32, tag="o_sb")
                        nc.vector.tensor_scalar_mul(o_sb, in0=opsum, scalar1=recip)
                        nc.vector.tensor_add(
                            o_sb, o_sb, bv_bc[:, jb * 512 : (jb + 1) * 512]
                        )
                        nc.sync.dma_start(
                            out_r[sb * (SBLK // 128) + su][
                                :, jb * 512 : (jb + 1) * 512
                            ],
                            o_sb,
                        )


def build_nc():
    global _CACHED_NC
    if _CACHED_NC is not None:
        return _CACHED_NC
    import concourse.tile as tile
    from concourse import bacc, mybir

    F32 = mybir.dt.float32
    F32R = mybir.dt.float32r
    nc = bacc.Bacc("TRN2", target_bir_lowering=False, debug=False)
    xt = nc.dram_tensor("xt", [D, S], F32R, kind="ExternalInput").ap()
    xn = nc.dram_tensor("xn", [S, D], F32R, kind="ExternalInput").ap()
    wq = nc.dram_tensor("wq", [128, DC, D], F32R, kind="ExternalInput").ap()
    wk = nc.dram_tensor("wk", [128, DC, D], F32R, kind="ExternalInput").ap()
    wv = nc.dram_tensor("wv", [128, DC, D], F32R, kind="ExternalInput").ap()
    bq = nc.dram_tensor("bq", [128, DC], F32, kind="ExternalInput").ap()
    bv = nc.dram_tensor("bv", [D], F32, kind="ExternalInput").ap()
    out = nc.dram_tensor("out", [HALF, D], F32, kind="ExternalOutput").ap()

    with tile.TileContext(nc) as tc:
        build_kernel(tc, xt, xn, wq, wk, wv, bq, bv, out)
    nc.compile()
    _CACHED_NC = nc
    return nc


def _shard_inputs(x, Wq, bq, Wk, bk, Wv, bv):
    """Host-side prep: per-core permuted x^T + relaid-out weights/biases."""
    wq_r = np.ascontiguousarray(Wq.reshape(DC, 128, D).transpose(1, 0, 2))
    wk_r = np.ascontiguousarray(Wk.T.reshape(DC, 128, D).transpose(1, 0, 2))
    wv_r = np.ascontiguousarray(Wv.reshape(DC, 128, D).transpose(1, 0, 2))
    bq_r = np.ascontiguousarray(bq.reshape(DC, 128).T)
    bv_c = np.ascontiguousarray(bv)

    in_maps = []
    for c in range(NC):
        b, h = divmod(c, 2)
        xb = x[b]
        if h:
            xb = np.concatenate([xb[HALF:], xb[:HALF]], axis=0)
        xt = np.ascontiguousarray(xb.T)  # [D, S], own queries first
        xn = np.ascontiguousarray(xb)  # [S, D], same permutation
        in_maps.append(
            {
                "xt": xt,
                "xn": xn,
                "wq": wq_r,
                "wk": wk_r,
                "wv": wv_r,
                "bq": bq_r,
                "bv": bv_c,
            }
        )
    return in_maps


def kernel(x, Wq, bq, Wk, bk, Wv, bv):
    global LAST_RESULT
    _ensure_axon_ntff_hook()
    from concourse import bass_utils

    x = np.asarray(x, dtype=np.float32)
    args = [np.asarray(a, dtype=np.float32) for a in (Wq, bq, Wk, bk, Wv, bv)]
    nc = build_nc()
    in_maps = _shard_inputs(x, *args)
    res = bass_utils.run_bass_kernel_spmd(nc, in_maps, core_ids=list(range(NC)))
    LAST_RESULT = res
    out = np.empty((B, S, D), dtype=np.float32)
    for c in range(NC):
        b, h = divmod(c, 2)
        out[b, h * HALF : (h + 1) * HALF, :] = res.results[c]["out"]
    return out


if __name__ == "__main__":
    rng = np.random.default_rng(0)
    init = 1.0 / 32.0
    x = rng.standard_normal((B, S, D), dtype=np.float32)
    mk = lambda *s: rng.uniform(-init, init, s).astype(np.float32)
    o = kernel(x, mk(D, D), mk(D), mk(D, D), mk(D), mk(D, D), mk(D))
    print("out", o.shape, o.dtype, float(np.abs(o).max()))
